# revision 1
# baseline (speedup 1.0000x reference)
"""Trainium2 Bass kernel for 2-layer GCN (nn_GCN_22866405884174).

Strategy (8 NeuronCores, dst-node sharding):
  out = A @ relu((A @ x) @ W1 + b1) @ W2 + b2   with A = D^-1/2 (Adj+I) D^-1/2
  (linear layers commute with aggregation, so each layer is: gather table
  rows by edge src + scatter-add by edge dst, then a small dense matmul).

  - Nodes sharded contiguously: core c owns dst nodes [c*12500, (c+1)*12500).
  - Host does graph preprocessing only (degrees, D^-1/2, edge sort, one-hot
    scatter blocks) per the sharding hint.
  - Layer tables are bf16 pre-scaled by dinv[src]; dinv[dst] applied
    post-aggregation on device. Scatter matrices S are exact one-hot fp8.
  - Gather: SWDGE dma_gather in 1024-index chunks (single_packet mode:
    64 descriptors per SDMA engine), issued prepare_only + trigger_dma so
    the Q7 only pays descriptor generation, not transfer/completion.
  - Scatter-add: PE matmul msg[slots,feat]^T @ S[slots,128dst] accumulated
    into a full PSUM bank [128, 512] per 8-window block (bank-wide
    has_written clear on the first matmul of the group).
  - Between layers: AllGather of the bf16 hidden table across the 8 cores.
"""

import numpy as np
import ml_dtypes

# ---------------- problem constants (hardcoded per contract) ----------------
N = 100000
E = 1600000
F_IN = 128
HID = 64
OUT_D = 10

NCORES = 8
NPC = N // NCORES           # 12500 nodes per core
SH = 12544                  # padded shard rows (98 * 128)
NTOT = SH * NCORES          # 100352
SEC = 25088                 # table section rows (2 shards, < int16 range)
NSEC = 4
WDST = 64                   # dst window width
NWIN = (NPC + WDST - 1) // WDST   # 196 (last window = 20 dst)
WB = 8                      # windows per block
NBLK = (NWIN + WB - 1) // WB      # 25 (last block = 4 windows)
BCOLS = WB * WDST           # 512 psum cols per block
NG = SH // 128              # 98 node groups per shard
SENTINEL = 12500            # zero pad row (same local idx in every section)
CHUNK = 1024                # gather chunk (single_packet limit: 16 engines x 64)

_CACHE = {}


# ============================ host preprocessing ============================

def _host_prep(edge_index):
    src = np.asarray(edge_index[0]).astype(np.int64)
    dst = np.asarray(edge_index[1]).astype(np.int64)
    loops = np.arange(N, dtype=np.int64)
    src = np.concatenate([src, loops])
    dst = np.concatenate([dst, loops])
    deg = np.bincount(dst, minlength=N).astype(np.float32)
    dinv = 1.0 / np.sqrt(deg)

    srow = (src // NPC) * SH + (src % NPC)
    core = dst // NPC
    dloc = dst % NPC
    win = dloc // WDST
    sec = srow // SEC

    cellid = (core * NWIN + win) * NSEC + sec
    counts = np.bincount(cellid, minlength=NCORES * NWIN * NSEC).reshape(NCORES, NWIN, NSEC)
    n_cell = counts.max(axis=0)
    n_cell = np.maximum(((n_cell + 15) // 16) * 16, 128)   # 16-aligned, >= 128

    # ---- schedule: section-major slot streams, (block,sec) runs 128-aligned ----
    # blocks[b][s] = {cells, run (128-mult), ngrp, groups:[(tile_k, jj, base)],
    #                 soff (slot offset within section stream), gi (S group off)}
    blocks = [[None] * NSEC for _ in range(NBLK)]
    sec_len = [0] * NSEC
    for s in range(NSEC):
        off = 0
        for b in range(NBLK):
            wlo, whi = b * WB, min(NWIN, (b + 1) * WB)
            cells = [int(n_cell[w, s]) for w in range(wlo, whi)]
            nbs = sum(cells)
            run = ((nbs + 127) // 128) * 128
            ngrp = run // 128
            bounds = np.cumsum([0] + cells)
            groups = []
            for j in range(ngrp):
                gslot = off + j * 128
                wi = int(np.searchsorted(bounds, j * 128, side="right") - 1)
                wi = min(wi, len(cells) - 1)
                base = min(wi * WDST, BCOLS - 128)
                groups.append((gslot // CHUNK, (gslot % CHUNK) // 128, base))
            blocks[b][s] = {
                "cells": cells, "nbs": nbs, "run": run, "ngrp": ngrp,
                "groups": groups, "soff": off,
            }
            off += run
        sec_len[s] = off

    # S group offsets in (b, s, j) order
    TG = 0
    for b in range(NBLK):
        for s in range(NSEC):
            blocks[b][s]["gi"] = TG
            TG += blocks[b][s]["ngrp"]

    # idx tensor: section streams concatenated
    sec_coff = [0] * NSEC
    CIDX = 0
    for s in range(NSEC):
        sec_coff[s] = CIDX
        CIDX += sec_len[s] // 16

    chunks = [[] for _ in range(NSEC)]   # per section: chunk sizes
    for s in range(NSEC):
        rem = sec_len[s]
        while rem > 0:
            chunks[s].append(min(CHUNK, rem))
            rem -= min(CHUNK, rem)

    sort_key = (sec + NSEC * (win + NWIN * core))
    order = np.lexsort((dloc, sort_key))
    srow_s = srow[order]
    dloc_s = dloc[order]
    key_s = sort_key[order]

    idx_all = np.zeros((NCORES, 128, CIDX), dtype=np.int16)
    sval_all = np.zeros((NCORES, TG, 128, 128), dtype=ml_dtypes.float8_e4m3)
    dinv_gt = np.zeros((NCORES, 128, NG), dtype=np.float32)

    cw_starts = np.searchsorted(key_s, np.arange(NCORES * NWIN * NSEC + 1))
    for c in range(NCORES):
        s_g = []
        s_p = []
        s_d = []
        for s in range(NSEC):
            stream = np.full(sec_len[s], SENTINEL, dtype=np.int64)
            dcol_st = np.full(sec_len[s], -1, dtype=np.int64)
            gblist = []
            for b in range(NBLK):
                info = blocks[b][s]
                off = info["soff"]
                wlo, whi = b * WB, min(NWIN, (b + 1) * WB)
                for wi, w in enumerate(range(wlo, whi)):
                    cid = (c * NWIN + w) * NSEC + s
                    a, e = cw_starts[cid], cw_starts[cid + 1]
                    cnt = e - a
                    stream[off:off + cnt] = srow_s[a:e] - s * SEC
                    dcol_st[off:off + cnt] = dloc_s[a:e] - b * BCOLS
                    off += info["cells"][wi]
                # S coords for this (b, s): slots [soff, soff+run)
                t0, t1 = info["soff"], info["soff"] + info["run"]
                t = np.arange(t0, t1)
                dc = dcol_st[t0:t1]
                real = dc >= 0
                j = (t - t0) // 128
                bases = np.array([g[2] for g in info["groups"]], dtype=np.int64)
                scol = dc - bases[j]
                if real.any():
                    assert scol[real].min() >= 0 and scol[real].max() < 128
                s_g.append(info["gi"] + j[real])
                s_p.append((t[real] - t0) % 128)
                s_d.append(scol[real])
            ncol = sec_len[s] // 16
            idx_all[c, :, sec_coff[s]:sec_coff[s] + ncol] = np.tile(
                stream.astype(np.int16).reshape(ncol, 16).T, (8, 1))
        sval_all[c, np.concatenate(s_g), np.concatenate(s_p),
                 np.concatenate(s_d)] = ml_dtypes.float8_e4m3(1.0)

        dpad = np.zeros(SH, dtype=np.float32)
        dpad[:NPC] = dinv[c * NPC:(c + 1) * NPC]
        dinv_gt[c] = dpad.reshape(NG, 128).T

    sched = {
        "blocks": blocks, "chunks": chunks, "sec_coff": sec_coff,
        "sec_len": sec_len, "CIDX": CIDX, "TG": TG,
    }
    return sched, idx_all, sval_all, dinv_gt, dinv


# ============================ device program ============================

def build_program(sched):
    import concourse.bass as bass
    import concourse.bacc as bacc
    import concourse.tile as tile
    import concourse.mybir as mybir

    CIDX = sched["CIDX"]
    TG = sched["TG"]
    blocks = sched["blocks"]
    chunks = sched["chunks"]
    sec_coff = sched["sec_coff"]
    GBLK_MAX = max(sum(blocks[b][s]["ngrp"] for s in range(NSEC)) for b in range(NBLK))

    nc = bacc.Bacc(None, target_bir_lowering=False, debug=False, num_swdge_queues=4)
    f32 = mybir.dt.float32
    bf16 = mybir.dt.bfloat16
    fp8 = mybir.dt.float8e4
    i16 = mybir.dt.int16

    T1 = nc.dram_tensor("T1", [NTOT, F_IN], bf16, kind="ExternalInput")
    IDX = nc.dram_tensor("IDX", [128, CIDX], i16, kind="ExternalInput")
    SVAL = nc.dram_tensor("SVAL", [TG, 128, 128], fp8, kind="ExternalInput")
    DINV = nc.dram_tensor("DINV", [128, NG], f32, kind="ExternalInput")
    W1T = nc.dram_tensor("W1T", [F_IN, HID], f32, kind="ExternalInput")
    B1T = nc.dram_tensor("B1T", [128, HID], f32, kind="ExternalInput")
    W2T = nc.dram_tensor("W2T", [HID, OUT_D], f32, kind="ExternalInput")
    B2T = nc.dram_tensor("B2T", [128, OUT_D], f32, kind="ExternalInput")
    OUTE = nc.dram_tensor("OUTE", [SH, OUT_D], f32, kind="ExternalOutput")

    t_local = nc.dram_tensor("t_local", [SH, F_IN], bf16)
    t_full = nc.dram_tensor("t_full", [NTOT, F_IN], bf16, addr_space="Shared")

    with tile.TileContext(nc) as tc:
        with (
            tc.tile_pool(name="resident", bufs=1) as rpool,
            tc.tile_pool(name="msg", bufs=20) as mpool,
            tc.tile_pool(name="sv", bufs=3) as spool,
            tc.tile_pool(name="post", bufs=4) as ppool,
            tc.tile_pool(name="psum", bufs=3, space="PSUM") as psum_pool,
            tc.tile_pool(name="psum2", bufs=4, space="PSUM") as psum_pool2,
        ):
            idx_t = rpool.tile([128, CIDX], i16)
            nc.sync.dma_start(idx_t[:], IDX[:])
            dinv_t = rpool.tile([128, NG], f32)
            nc.sync.dma_start(dinv_t[:], DINV[:])
            w1_t = rpool.tile([F_IN, HID], f32)
            nc.sync.dma_start(w1_t[:], W1T[:])
            b1_t = rpool.tile([128, HID], f32)
            nc.sync.dma_start(b1_t[:], B1T[:])
            w2_t = rpool.tile([HID, OUT_D], f32)
            nc.sync.dma_start(w2_t[:], W2T[:])
            b2_t = rpool.tile([128, OUT_D], f32)
            nc.sync.dma_start(b2_t[:], B2T[:])

            agg1 = rpool.tile([128, SH], f32)
            agg2 = rpool.tile([HID, SH], f32)

            def scatter_layer(table, agg):
                # issue gathers round-robin over sections, chunk index k;
                # 4-way queue parallelism across the Q7 core pairs
                msg_tiles = {}
                qn = 0
                kmax = max(len(chunks[s]) for s in range(NSEC))
                for k in range(kmax):
                    for s in range(NSEC):
                        if k >= len(chunks[s]):
                            continue
                        csz = chunks[s][k]
                        ng = (csz + 127) // 128
                        msg = mpool.tile([128, CHUNK // 128, F_IN], bf16, tag="msg")
                        nc.gpsimd.dma_gather(
                            msg[:, :ng, :],
                            table[s * SEC:(s + 1) * SEC, :],
                            idx_t[:, sec_coff[s] + k * (CHUNK // 16):
                                  sec_coff[s] + k * (CHUNK // 16) + csz // 16],
                            csz,
                            csz,
                            F_IN,
                            single_packet=True,
                            queue_num=qn,
                        )
                        qn = (qn + 1) % 4
                        msg_tiles[(s, k)] = msg
                for b in range(NBLK):
                    blo = b * BCOLS
                    blen = min(BCOLS, NPC - blo)
                    gblk = sum(blocks[b][s]["ngrp"] for s in range(NSEC))
                    g0 = blocks[b][0]["gi"]
                    s_t = spool.tile([128, GBLK_MAX, 128], fp8, tag="sval")
                    nc.sync.dma_start(
                        s_t[:, :gblk, :],
                        SVAL[g0:g0 + gblk].rearrange("g p w -> p g w"),
                    )
                    acc = psum_pool.tile([128, BCOLS], f32, tag="acc")
                    mm = 0
                    for s in range(NSEC):
                        info = blocks[b][s]
                        for (tk, jj, base) in info["groups"]:
                            nc.tensor.matmul(
                                acc[:, base:base + 128],
                                msg_tiles[(s, tk)][:, jj, :],
                                s_t[:, mm, :],
                                start=(mm == 0),
                                stop=(mm == gblk - 1),
                            )
                            mm += 1
                    nc.vector.tensor_copy(
                        agg[:, blo:blo + blen], acc[:agg.shape[0], :blen]
                    )
                nc.vector.memset(agg[:, NPC:SH], 0.0)

            # ---------------- layer 1 ----------------
            scatter_layer(T1, agg1)

            for g in range(NG):
                ph = psum_pool2.tile([128, HID], f32, tag="wout")
                nc.tensor.matmul(
                    ph[:], agg1[:, g * 128:(g + 1) * 128], w1_t[:],
                    start=True, stop=True,
                )
                tmp = ppool.tile([128, HID], f32, tag="tmp")
                nc.vector.tensor_scalar(
                    out=tmp[:], in0=ph[:], scalar1=dinv_t[:, g:g + 1],
                    scalar2=None, op0=mybir.AluOpType.mult,
                )
                nc.vector.tensor_tensor(
                    out=tmp[:], in0=tmp[:], in1=b1_t[:], op=mybir.AluOpType.add
                )
                t_out = ppool.tile([128, F_IN], bf16, tag="tout")
                nc.vector.memset(t_out[:, HID:], 0.0)
                nc.vector.tensor_scalar(
                    out=t_out[:, :HID], in0=tmp[:], scalar1=0.0,
                    scalar2=dinv_t[:, g:g + 1], op0=mybir.AluOpType.max,
                    op1=mybir.AluOpType.mult,
                )
                nc.sync.dma_start(t_local[g * 128:(g + 1) * 128, :], t_out[:])

            nc.gpsimd.collective_compute(
                "AllGather",
                mybir.AluOpType.bypass,
                replica_groups=[list(range(NCORES))],
                ins=[t_local[:]],
                outs=[t_full[:]],
            )

            # ---------------- layer 2 ----------------
            scatter_layer(t_full, agg2)

            for g in range(NG):
                po = psum_pool2.tile([128, OUT_D], f32, tag="wout")
                nc.tensor.matmul(
                    po[:], agg2[:, g * 128:(g + 1) * 128], w2_t[:],
                    start=True, stop=True,
                )
                ot = ppool.tile([128, OUT_D], f32, tag="ot")
                nc.vector.tensor_scalar(
                    out=ot[:], in0=po[:], scalar1=dinv_t[:, g:g + 1],
                    scalar2=None, op0=mybir.AluOpType.mult,
                )
                nc.vector.tensor_tensor(
                    out=ot[:], in0=ot[:], in1=b2_t[:], op=mybir.AluOpType.add
                )
                nc.sync.dma_start(OUTE[g * 128:(g + 1) * 128, :], ot[:])

    nc.compile()
    return nc


# ============================ entry point ============================

def prepare(x, edge_index, W1, b1, W2, b2):
    x = np.asarray(x, dtype=np.float32)
    W1 = np.asarray(W1, dtype=np.float32)
    b1 = np.asarray(b1, dtype=np.float32)
    W2 = np.asarray(W2, dtype=np.float32)
    b2 = np.asarray(b2, dtype=np.float32)

    sched, idx_all, sval_all, dinv_gt, dinv = _host_prep(edge_index)

    key = ("v4", sched["CIDX"], sched["TG"])
    if key in _CACHE:
        nc = _CACHE[key]
    else:
        nc = build_program(sched)
        _CACHE[key] = nc

    xs = x * dinv[:, None]
    T1 = np.zeros((NTOT, F_IN), dtype=ml_dtypes.bfloat16)
    for c in range(NCORES):
        T1[c * SH:c * SH + NPC] = xs[c * NPC:(c + 1) * NPC].astype(ml_dtypes.bfloat16)

    b1_tile = np.tile(b1[None, :], (128, 1)).astype(np.float32)
    b2_tile = np.tile(b2[None, :], (128, 1)).astype(np.float32)

    in_maps = []
    for c in range(NCORES):
        in_maps.append({
            "T1": T1,
            "IDX": np.ascontiguousarray(idx_all[c]),
            "SVAL": np.ascontiguousarray(sval_all[c]),
            "DINV": np.ascontiguousarray(dinv_gt[c]),
            "W1T": W1,
            "B1T": b1_tile,
            "W2T": W2,
            "B2T": b2_tile,
        })
    return nc, in_maps


def kernel(x, edge_index, W1, b1, W2, b2):
    from concourse.bass_utils import run_bass_kernel_spmd

    nc, in_maps = prepare(x, edge_index, W1, b1, W2, b2)
    r = run_bass_kernel_spmd(nc, in_maps, core_ids=list(range(NCORES)))
    out = np.empty((N, OUT_D), dtype=np.float32)
    for c in range(NCORES):
        out[c * NPC:(c + 1) * NPC] = r.results[c]["OUTE"][:NPC]
    return out



# revision 6
# speedup vs baseline: 1.7029x; 1.7029x over previous
"""Trainium2 Bass kernel for 2-layer GCN (nn_GCN_22866405884174).

v2 strategy (8 NeuronCores, dst-node sharding):
  out = A @ relu((A @ x) @ W1 + b1) @ W2 + b2   with A = D^-1/2 (Adj+I) D^-1/2

  Each layer = per-edge gather (SWDGE dma_gather) of pre-scaled source rows,
  one-hot fp8 scatter matmul into a PSUM block [*, 512 dst], then a small
  dense transform per 128-dst group.

  v2 changes vs v1 (trace-driven):
  - Self-loops removed from the edge stream; the diagonal term is added
    densely from transposed local tables (T1D input / on-device PE
    transpose of the hidden layer).
  - Exact per-core edge streams: slots per (sec, block) cell padded only to
    the max across cores rounded to 128 (quantile-spread groups, per-group
    column bases chosen host-side) -- kills the per-window 16-align+128-min
    padding of v1 (~-20% descriptors).
  - SVAL stored pre-transposed [128, TG, 128] so each per-block load is one
    contiguous 2D DMA (v1's rearrange cost ~1.5ms of HWDGE gen on Sync).
  - Tables stored partition-major (row = c*SH + p*NG + g) so t_local and
    OUTE are written with a single contiguous DMA each instead of 98 2D
    DMAs per layer.
  - Layer-2 scatter matmuls use M=64 (only the real hidden columns).
  - Larger gather chunks to amortize the ~1us fixed SWDGE cost per
    instruction (descriptor generation on the Q7 is the serial bottleneck).
"""

import numpy as np
import ml_dtypes

# ---------------- problem constants (hardcoded per contract) ----------------
N = 100000
E = 1600000
F_IN = 128
HID = 64
OUT_D = 10

NCORES = 8
NPC = N // NCORES           # 12500 nodes per core
NG = 98                     # node groups of 128 per shard
SH = NG * 128               # 12544 padded shard rows
NTOT = SH * NCORES          # 100352
SEC = 25088                 # table section rows (2 shards, < int16 range)
NSEC = 4
BCOLS = 512                 # psum block width (dst cols)
NBLK = (NPC + BCOLS - 1) // BCOLS   # 25 (last block = 212 dst)
CHUNK = 1024                # gather chunk (tunable; single_packet limit 1024)
MSGBUF = 12                 # in-flight gather chunk tiles
SENT_LOCAL = 84 * NG + 97   # zero pad row (p=84, g=97 -> node 12500), per section

_CACHE = {}


# ============================ host preprocessing ============================

def _round128(v):
    return ((int(v) + 127) // 128) * 128


def _host_prep(edge_index):
    src = np.asarray(edge_index[0]).astype(np.int64)
    dst = np.asarray(edge_index[1]).astype(np.int64)
    deg = (np.bincount(dst, minlength=N) + 1.0).astype(np.float32)  # + self loop
    dinv = (1.0 / np.sqrt(deg)).astype(np.float32)

    core = dst // NPC
    dloc = dst % NPC
    blk = dloc // BCOLS
    sl = src % NPC
    srow = (src // NPC) * SH + (sl % 128) * NG + (sl // 128)
    sec = srow // SEC

    order = np.lexsort((dloc, blk, sec, core))
    srow_s = srow[order]
    dloc_s = dloc[order]
    cell = ((core * NSEC + sec) * NBLK + blk)[order]
    starts = np.searchsorted(cell, np.arange(NCORES * NSEC * NBLK + 1))
    cnt = np.diff(starts).reshape(NCORES, NSEC, NBLK)

    # --- per (sec, blk): run size + group column bases (shared across cores) ---
    run = np.zeros((NSEC, NBLK), np.int64)
    bases = [[None] * NBLK for _ in range(NSEC)]
    for s in range(NSEC):
        for b in range(NBLK):
            r = max(_round128(cnt[:, s, b].max()), 128)
            while True:
                ngrp = r // 128
                lo = np.full(ngrp, BCOLS, np.int64)
                hi = np.full(ngrp, -1, np.int64)
                for c in range(NCORES):
                    m = cnt[c, s, b]
                    if m == 0:
                        continue
                    a = starts[(c * NSEC + s) * NBLK + b]
                    dc = dloc_s[a:a + m] - b * BCOLS
                    tj = (np.arange(ngrp + 1) * m) // ngrp
                    for j in range(ngrp):
                        if tj[j + 1] > tj[j]:
                            lo[j] = min(lo[j], dc[tj[j]])
                            hi[j] = max(hi[j], dc[tj[j + 1] - 1])
                bj = np.clip(lo, 0, BCOLS - 128)
                if np.all(hi < bj + 128):
                    bases[s][b] = bj
                    run[s, b] = r
                    break
                r += 128
    soff = np.zeros((NSEC, NBLK), np.int64)
    sec_len = np.zeros(NSEC, np.int64)
    for s in range(NSEC):
        soff[s] = np.concatenate([[0], np.cumsum(run[s])[:-1]])
        sec_len[s] = run[s].sum()

    sec_coff = np.concatenate([[0], np.cumsum(sec_len // 16)[:-1]]).astype(np.int64)
    CIDX = int((sec_len // 16).sum())

    chunks = []
    for s in range(NSEC):
        rem = int(sec_len[s])
        cs = []
        while rem > 0:
            cs.append(min(CHUNK, rem))
            rem -= cs[-1]
        chunks.append(cs)

    # program group order: block-major (b, s, j)
    gi0 = np.zeros((NBLK, NSEC), np.int64)
    TG = 0
    groups = [[None] * NSEC for _ in range(NBLK)]
    for b in range(NBLK):
        for s in range(NSEC):
            gi0[b, s] = TG
            ng = int(run[s, b]) // 128
            gl = []
            for j in range(ng):
                gslot = int(soff[s, b]) + j * 128
                gl.append((gslot // CHUNK, (gslot % CHUNK) // 128,
                           int(bases[s][b][j])))
            groups[b][s] = gl
            TG += ng

    # --- per-core streams + S values ---
    idx_all = np.zeros((NCORES, 128, CIDX), dtype=np.int16)
    sval_all = np.zeros((NCORES, 128, TG, 128), dtype=ml_dtypes.float8_e4m3)
    dinv_gt = np.zeros((NCORES, 128, NG), dtype=np.float32)

    for c in range(NCORES):
        p_l, g_l, w_l = [], [], []
        for s in range(NSEC):
            stream = np.full(int(sec_len[s]), SENT_LOCAL, dtype=np.int16)
            for b in range(NBLK):
                m = cnt[c, s, b]
                if m == 0:
                    continue
                a = starts[(c * NSEC + s) * NBLK + b]
                sr = srow_s[a:a + m] - s * SEC
                dc = dloc_s[a:a + m] - b * BCOLS
                ngrp = int(run[s, b]) // 128
                tj = (np.arange(ngrp + 1) * m) // ngrp
                t = np.arange(m)
                j = np.searchsorted(tj, t, side="right") - 1
                p = t - tj[j]
                pos = int(soff[s, b]) + j * 128 + p
                stream[pos] = sr.astype(np.int16)
                w = dc - bases[s][b][j]
                assert w.min() >= 0 and w.max() < 128
                p_l.append(p)
                g_l.append(gi0[b, s] + j)
                w_l.append(w)
            ncol = int(sec_len[s]) // 16
            idx_all[c, :, sec_coff[s]:sec_coff[s] + ncol] = np.tile(
                stream.reshape(ncol, 16).T, (8, 1))
        sval_all[c, np.concatenate(p_l), np.concatenate(g_l),
                 np.concatenate(w_l)] = ml_dtypes.float8_e4m3(1.0)

        dpad = np.zeros(SH, dtype=np.float32)
        dpad[:NPC] = dinv[c * NPC:(c + 1) * NPC]
        dinv_gt[c] = dpad.reshape(NG, 128).T

    sched = {
        "groups": groups, "chunks": chunks, "sec_coff": sec_coff,
        "sec_len": sec_len, "CIDX": CIDX, "TG": TG,
    }
    return sched, idx_all, sval_all, dinv_gt, dinv


# ============================ device program ============================

def build_program(sched):
    import concourse.bass as bass  # noqa: F401
    import concourse.bacc as bacc
    import concourse.tile as tile
    import concourse.mybir as mybir
    from concourse.masks import make_identity

    CIDX = sched["CIDX"]
    TG = sched["TG"]
    groups = sched["groups"]
    chunks = sched["chunks"]
    sec_coff = sched["sec_coff"]
    GBLK_MAX = max(sum(len(groups[b][s]) for s in range(NSEC)) for b in range(NBLK))

    nc = bacc.Bacc(None, target_bir_lowering=False, debug=False, num_swdge_queues=4)
    f32 = mybir.dt.float32
    bf16 = mybir.dt.bfloat16
    fp8 = mybir.dt.float8e4
    i16 = mybir.dt.int16

    T1 = nc.dram_tensor("T1", [NTOT, F_IN], bf16, kind="ExternalInput")
    IDX = nc.dram_tensor("IDX", [128, CIDX], i16, kind="ExternalInput")
    SVAL = nc.dram_tensor("SVAL", [128, TG, 128], fp8, kind="ExternalInput")
    DINV = nc.dram_tensor("DINV", [128, NG], f32, kind="ExternalInput")
    T1D = nc.dram_tensor("T1D", [128, NG * 128], bf16, kind="ExternalInput")
    W1T = nc.dram_tensor("W1T", [F_IN, HID], f32, kind="ExternalInput")
    B1T = nc.dram_tensor("B1T", [128, HID], f32, kind="ExternalInput")
    W2T = nc.dram_tensor("W2T", [HID, OUT_D], f32, kind="ExternalInput")
    B2T = nc.dram_tensor("B2T", [128, OUT_D], f32, kind="ExternalInput")
    OUTE = nc.dram_tensor("OUTE", [128, NG * OUT_D], f32, kind="ExternalOutput")

    t_local = nc.dram_tensor("t_local", [128, NG * F_IN], bf16)
    t_full = nc.dram_tensor("t_full", [NTOT, F_IN], bf16, addr_space="Shared")

    with tile.TileContext(nc) as tc:
        with (
            tc.tile_pool(name="resident", bufs=1) as rpool,
            tc.tile_pool(name="msg", bufs=MSGBUF) as mpool,
            tc.tile_pool(name="sv", bufs=3) as spool,
            tc.tile_pool(name="post", bufs=3) as ppool,
            tc.tile_pool(name="tiny", bufs=3) as ypool,
            tc.tile_pool(name="acc", bufs=2, space="PSUM") as apool,
            tc.tile_pool(name="tp", bufs=2, space="PSUM") as tpool,
        ):
            idx_t = rpool.tile([128, CIDX], i16)
            nc.sync.dma_start(idx_t[:], IDX[:])
            dinv_t = rpool.tile([128, NG], f32)
            nc.sync.dma_start(dinv_t[:], DINV[:])
            w1_t = rpool.tile([F_IN, HID], f32)
            nc.sync.dma_start(w1_t[:], W1T[:])
            b1_t = rpool.tile([128, HID], f32)
            nc.sync.dma_start(b1_t[:], B1T[:])
            w2_t = rpool.tile([HID, OUT_D], f32)
            nc.sync.dma_start(w2_t[:], W2T[:])
            b2_t = rpool.tile([128, OUT_D], f32)
            nc.sync.dma_start(b2_t[:], B2T[:])
            t1d_t = rpool.tile([128, NG * 128], bf16)
            nc.sync.dma_start(t1d_t[:], T1D[:])

            ident = rpool.tile([128, 128], bf16)
            make_identity(nc, ident[:])

            tloc_t = rpool.tile([128, NG * F_IN], bf16)
            nc.vector.memset(tloc_t[:], 0.0)
            ht_t = rpool.tile([HID, NG * 128], bf16)
            oute_t = rpool.tile([128, NG * OUT_D], f32)

            def issue_gathers(table):
                msg_tiles = {}
                qn = 0
                kmax = max(len(cs) for cs in chunks)
                for k in range(kmax):
                    for s in range(NSEC):
                        if k >= len(chunks[s]):
                            continue
                        csz = chunks[s][k]
                        ng = csz // 128
                        msg = mpool.tile([128, CHUNK // 128, F_IN], bf16, tag="msg")
                        nc.gpsimd.dma_gather(
                            msg[:, :ng, :],
                            table[s * SEC:(s + 1) * SEC, :],
                            idx_t[:, sec_coff[s] + k * (CHUNK // 16):
                                  sec_coff[s] + k * (CHUNK // 16) + csz // 16],
                            csz,
                            csz,
                            F_IN,
                            single_packet=True,
                            queue_num=qn,
                        )
                        qn = (qn + 1) % 4
                        msg_tiles[(s, k)] = msg
                return msg_tiles

            def scatter_block(b, msg_tiles, macc, mwid):
                gblk = sum(len(groups[b][s]) for s in range(NSEC))
                g0 = sum(sum(len(groups[bb][s]) for s in range(NSEC))
                         for bb in range(b))
                s_t = spool.tile([128, GBLK_MAX, 128], fp8, tag="sval")
                nc.sync.dma_start(s_t[:, :gblk, :], SVAL[:, g0:g0 + gblk, :])
                acc = apool.tile([macc, BCOLS], f32, tag=f"acc{macc}")
                mm = 0
                for s in range(NSEC):
                    for (tk, jj, base) in groups[b][s]:
                        nc.tensor.matmul(
                            acc[:, base:base + 128],
                            msg_tiles[(s, tk)][:, jj, :mwid],
                            s_t[:, mm, :],
                            start=(mm == 0),
                            stop=(mm == gblk - 1),
                        )
                        mm += 1
                return acc

            # ---------------- layer 1 ----------------
            msg_tiles = issue_gathers(T1)
            for b in range(NBLK):
                blo = b * BCOLS
                blen = min(BCOLS, NPC - blo)
                acc = scatter_block(b, msg_tiles, 128, 128)
                hb = ppool.tile([128, BCOLS], f32, tag="hb1")
                nc.vector.tensor_tensor(
                    out=hb[:, :blen], in0=acc[:, :blen],
                    in1=t1d_t[:, blo:blo + blen], op=mybir.AluOpType.add)
                if blen < BCOLS:
                    nc.vector.memset(hb[:, blen:_round128(blen)], 0.0)
                for g in range(blo // 128, (blo + blen + 127) // 128):
                    go = g * 128 - blo
                    pt = tpool.tile([128, 128], f32, tag="t")
                    nc.tensor.matmul(
                        pt[:, :HID], hb[:, go:go + 128], w1_t[:],
                        start=True, stop=True)
                    tmp = ypool.tile([128, HID], f32, tag="tmp")
                    nc.vector.tensor_scalar(
                        out=tmp[:], in0=pt[:, :HID], scalar1=dinv_t[:, g:g + 1],
                        scalar2=None, op0=mybir.AluOpType.mult)
                    nc.vector.tensor_tensor(
                        out=tmp[:], in0=tmp[:], in1=b1_t[:],
                        op=mybir.AluOpType.add)
                    # relu + src-side dinv pre-scale, written into the table
                    plim = 84 if g == NG - 1 else 128
                    nc.vector.tensor_scalar(
                        out=tloc_t[:plim, g * F_IN:g * F_IN + HID],
                        in0=tmp[:plim, :], scalar1=0.0,
                        scalar2=dinv_t[:plim, g:g + 1],
                        op0=mybir.AluOpType.max, op1=mybir.AluOpType.mult)
                    # transposed copy for the layer-2 diagonal term
                    pt2 = tpool.tile([128, 128], bf16, tag="tb")
                    nc.tensor.transpose(
                        pt2[:HID, :], tloc_t[:, g * F_IN:g * F_IN + HID],
                        ident[:])
                    nc.vector.tensor_copy(
                        ht_t[:, g * 128:(g + 1) * 128], pt2[:HID, :])

            nc.sync.dma_start(t_local[:], tloc_t[:])
            nc.gpsimd.collective_compute(
                "AllGather",
                mybir.AluOpType.bypass,
                replica_groups=[list(range(NCORES))],
                ins=[t_local[:]],
                outs=[t_full[:]],
            )

            # ---------------- layer 2 ----------------
            msg_tiles = issue_gathers(t_full)
            for b in range(NBLK):
                blo = b * BCOLS
                blen = min(BCOLS, NPC - blo)
                acc = scatter_block(b, msg_tiles, HID, HID)
                hb = ppool.tile([HID, BCOLS], f32, tag="hb2")
                nc.vector.tensor_tensor(
                    out=hb[:, :blen], in0=acc[:, :blen],
                    in1=ht_t[:, blo:blo + blen], op=mybir.AluOpType.add)
                if blen < BCOLS:
                    nc.vector.memset(hb[:, blen:_round128(blen)], 0.0)
                for g in range(blo // 128, (blo + blen + 127) // 128):
                    go = g * 128 - blo
                    pt = tpool.tile([128, 128], f32, tag="t")
                    nc.tensor.matmul(
                        pt[:, :OUT_D], hb[:, go:go + 128], w2_t[:],
                        start=True, stop=True)
                    ot = ypool.tile([128, OUT_D], f32, tag="ot")
                    nc.vector.tensor_scalar(
                        out=ot[:], in0=pt[:, :OUT_D], scalar1=dinv_t[:, g:g + 1],
                        scalar2=None, op0=mybir.AluOpType.mult)
                    nc.vector.tensor_tensor(
                        out=oute_t[:, g * OUT_D:(g + 1) * OUT_D],
                        in0=ot[:], in1=b2_t[:], op=mybir.AluOpType.add)

            nc.sync.dma_start(OUTE[:], oute_t[:])

    nc.compile()
    return nc


# ============================ entry point ============================

def prepare(x, edge_index, W1, b1, W2, b2):
    x = np.asarray(x, dtype=np.float32)
    W1 = np.asarray(W1, dtype=np.float32)
    b1 = np.asarray(b1, dtype=np.float32)
    W2 = np.asarray(W2, dtype=np.float32)
    b2 = np.asarray(b2, dtype=np.float32)

    sched, idx_all, sval_all, dinv_gt, dinv = _host_prep(edge_index)

    key = ("v2", CHUNK, sched["CIDX"], sched["TG"])
    if key in _CACHE:
        nc = _CACHE[key]
    else:
        nc = build_program(sched)
        _CACHE[key] = nc

    xs = x * dinv[:, None]
    T1 = np.zeros((NTOT, F_IN), dtype=ml_dtypes.bfloat16)
    n = np.arange(N)
    sl = n % NPC
    srow = (n // NPC) * SH + (sl % 128) * NG + (sl // 128)
    T1[srow] = xs.astype(ml_dtypes.bfloat16)

    b1_tile = np.tile(b1[None, :], (128, 1)).astype(np.float32)
    b2_tile = np.tile(b2[None, :], (128, 1)).astype(np.float32)

    in_maps = []
    for c in range(NCORES):
        t1d = np.zeros((128, NG * 128), dtype=ml_dtypes.bfloat16)
        xsl = np.zeros((SH, F_IN), dtype=np.float32)
        xsl[:NPC] = xs[c * NPC:(c + 1) * NPC]
        # t1d[f, l] = xs[c*NPC + l][f]
        t1d[:, :] = xsl.T.astype(ml_dtypes.bfloat16)
        in_maps.append({
            "T1": T1,
            "IDX": np.ascontiguousarray(idx_all[c]),
            "SVAL": np.ascontiguousarray(sval_all[c]),
            "DINV": np.ascontiguousarray(dinv_gt[c]),
            "T1D": t1d,
            "W1T": W1,
            "B1T": b1_tile,
            "W2T": W2,
            "B2T": b2_tile,
        })
    return nc, in_maps


def assemble_output(results):
    out = np.empty((N, OUT_D), dtype=np.float32)
    for c in range(NCORES):
        oc = np.asarray(results[c]["OUTE"]).reshape(128, NG, OUT_D)
        out[c * NPC:(c + 1) * NPC] = (
            oc.transpose(1, 0, 2).reshape(SH, OUT_D)[:NPC])
    return out


def kernel(x, edge_index, W1, b1, W2, b2):
    from concourse.bass_utils import run_bass_kernel_spmd

    nc, in_maps = prepare(x, edge_index, W1, b1, W2, b2)
    r = run_bass_kernel_spmd(nc, in_maps, core_ids=list(range(NCORES)))
    return assemble_output(r.results)


# revision 12
# speedup vs baseline: 1.7390x; 1.0212x over previous
"""Trainium2 Bass kernel for 2-layer GCN (nn_GCN_22866405884174).

v2 strategy (8 NeuronCores, dst-node sharding):
  out = A @ relu((A @ x) @ W1 + b1) @ W2 + b2   with A = D^-1/2 (Adj+I) D^-1/2

  Each layer = per-edge gather (SWDGE dma_gather) of pre-scaled source rows,
  one-hot fp8 scatter matmul into a PSUM block [*, 512 dst], then a small
  dense transform per 128-dst group.

  v2 changes vs v1 (trace-driven):
  - Self-loops removed from the edge stream; the diagonal term is added
    densely from transposed local tables (T1D input / on-device PE
    transpose of the hidden layer).
  - Exact per-core edge streams: slots per (sec, block) cell padded only to
    the max across cores rounded to 128 (quantile-spread groups, per-group
    column bases chosen host-side) -- kills the per-window 16-align+128-min
    padding of v1 (~-20% descriptors).
  - SVAL stored pre-transposed [128, TG, 128] so each per-block load is one
    contiguous 2D DMA (v1's rearrange cost ~1.5ms of HWDGE gen on Sync).
  - Tables stored partition-major (row = c*SH + p*NG + g) so t_local and
    OUTE are written with a single contiguous DMA each instead of 98 2D
    DMAs per layer.
  - Layer-2 scatter matmuls use M=64 (only the real hidden columns).
  - Larger gather chunks to amortize the ~1us fixed SWDGE cost per
    instruction (descriptor generation on the Q7 is the serial bottleneck).
"""

import numpy as np
import ml_dtypes

# ---------------- problem constants (hardcoded per contract) ----------------
N = 100000
E = 1600000
F_IN = 128
HID = 64
OUT_D = 10

NCORES = 8
NPC = N // NCORES           # 12500 nodes per core
NG = 98                     # node groups of 128 per shard
SH = NG * 128               # 12544 padded shard rows
NTOT = SH * NCORES          # 100352
SEC = 25088                 # table section rows (2 shards, < int16 range)
NSEC = 4
BCOLS = 512                 # psum block width (dst cols)
NBLK = (NPC + BCOLS - 1) // BCOLS   # 25 (last block = 212 dst)
CHUNK = 1024                # gather chunk (tunable; single_packet limit 1024)
MSGBUF = 32                 # in-flight gather chunk tiles
PREPN = 32                  # layer-2 chunks prepped during the AllGather
SENT_LOCAL = 84 * NG + 97   # zero pad row (p=84, g=97 -> node 12500), per section

_CACHE = {}


# ============================ host preprocessing ============================

def _round128(v):
    return ((int(v) + 127) // 128) * 128


def _host_prep(edge_index):
    src = np.asarray(edge_index[0]).astype(np.int64)
    dst = np.asarray(edge_index[1]).astype(np.int64)
    deg = (np.bincount(dst, minlength=N) + 1.0).astype(np.float32)  # + self loop
    dinv = (1.0 / np.sqrt(deg)).astype(np.float32)

    core = dst // NPC
    dloc = dst % NPC
    blk = dloc // BCOLS
    sl = src % NPC
    srow = (src // NPC) * SH + (sl % 128) * NG + (sl // 128)
    sec = srow // SEC

    order = np.lexsort((dloc, blk, sec, core))
    srow_s = srow[order]
    dloc_s = dloc[order]
    cell = ((core * NSEC + sec) * NBLK + blk)[order]
    starts = np.searchsorted(cell, np.arange(NCORES * NSEC * NBLK + 1))
    cnt = np.diff(starts).reshape(NCORES, NSEC, NBLK)

    # --- per (sec, blk): run size + group column bases (shared across cores) ---
    run = np.zeros((NSEC, NBLK), np.int64)
    bases = [[None] * NBLK for _ in range(NSEC)]
    for s in range(NSEC):
        for b in range(NBLK):
            r = max(_round128(cnt[:, s, b].max()), 128)
            while True:
                ngrp = r // 128
                lo = np.full(ngrp, BCOLS, np.int64)
                hi = np.full(ngrp, -1, np.int64)
                for c in range(NCORES):
                    m = cnt[c, s, b]
                    if m == 0:
                        continue
                    a = starts[(c * NSEC + s) * NBLK + b]
                    dc = dloc_s[a:a + m] - b * BCOLS
                    tj = (np.arange(ngrp + 1) * m) // ngrp
                    for j in range(ngrp):
                        if tj[j + 1] > tj[j]:
                            lo[j] = min(lo[j], dc[tj[j]])
                            hi[j] = max(hi[j], dc[tj[j + 1] - 1])
                bj = np.clip(lo, 0, BCOLS - 128)
                if np.all(hi < bj + 128):
                    bases[s][b] = bj
                    run[s, b] = r
                    break
                r += 128
    soff = np.zeros((NSEC, NBLK), np.int64)
    sec_len = np.zeros(NSEC, np.int64)
    for s in range(NSEC):
        soff[s] = np.concatenate([[0], np.cumsum(run[s])[:-1]])
        sec_len[s] = run[s].sum()

    sec_coff = np.concatenate([[0], np.cumsum(sec_len // 16)[:-1]]).astype(np.int64)
    CIDX = int((sec_len // 16).sum())

    chunks = []
    for s in range(NSEC):
        rem = int(sec_len[s])
        cs = []
        while rem > 0:
            cs.append(min(CHUNK, rem))
            rem -= cs[-1]
        chunks.append(cs)

    # program group order: block-major (b, s, j)
    gi0 = np.zeros((NBLK, NSEC), np.int64)
    TG = 0
    groups = [[None] * NSEC for _ in range(NBLK)]
    for b in range(NBLK):
        for s in range(NSEC):
            gi0[b, s] = TG
            ng = int(run[s, b]) // 128
            gl = []
            for j in range(ng):
                gslot = int(soff[s, b]) + j * 128
                gl.append((gslot // CHUNK, (gslot % CHUNK) // 128,
                           int(bases[s][b][j])))
            groups[b][s] = gl
            TG += ng

    # --- per-core streams + S values ---
    idx_all = np.zeros((NCORES, 128, CIDX), dtype=np.int16)
    sval_all = np.zeros((NCORES, 128, TG, 128), dtype=ml_dtypes.float8_e4m3)
    dinv_gt = np.zeros((NCORES, 128, NG), dtype=np.float32)

    for c in range(NCORES):
        p_l, g_l, w_l = [], [], []
        for s in range(NSEC):
            stream = np.full(int(sec_len[s]), SENT_LOCAL, dtype=np.int16)
            for b in range(NBLK):
                m = cnt[c, s, b]
                if m == 0:
                    continue
                a = starts[(c * NSEC + s) * NBLK + b]
                sr = srow_s[a:a + m] - s * SEC
                dc = dloc_s[a:a + m] - b * BCOLS
                ngrp = int(run[s, b]) // 128
                tj = (np.arange(ngrp + 1) * m) // ngrp
                t = np.arange(m)
                j = np.searchsorted(tj, t, side="right") - 1
                p = t - tj[j]
                pos = int(soff[s, b]) + j * 128 + p
                stream[pos] = sr.astype(np.int16)
                w = dc - bases[s][b][j]
                assert w.min() >= 0 and w.max() < 128
                p_l.append(p)
                g_l.append(gi0[b, s] + j)
                w_l.append(w)
            ncol = int(sec_len[s]) // 16
            idx_all[c, :, sec_coff[s]:sec_coff[s] + ncol] = np.tile(
                stream.reshape(ncol, 16).T, (8, 1))
        sval_all[c, np.concatenate(p_l), np.concatenate(g_l),
                 np.concatenate(w_l)] = ml_dtypes.float8_e4m3(1.0)

        dpad = np.zeros(SH, dtype=np.float32)
        dpad[:NPC] = dinv[c * NPC:(c + 1) * NPC]
        dinv_gt[c] = dpad.reshape(NG, 128).T

    sched = {
        "groups": groups, "chunks": chunks, "sec_coff": sec_coff,
        "sec_len": sec_len, "CIDX": CIDX, "TG": TG,
    }
    return sched, idx_all, sval_all, dinv_gt, dinv


# ============================ device program ============================

def build_program(sched):
    import concourse.bass as bass  # noqa: F401
    import concourse.bacc as bacc
    import concourse.tile as tile
    import concourse.mybir as mybir
    from concourse.masks import make_identity

    CIDX = sched["CIDX"]
    TG = sched["TG"]
    groups = sched["groups"]
    chunks = sched["chunks"]
    sec_coff = sched["sec_coff"]
    GBLK_MAX = max(sum(len(groups[b][s]) for s in range(NSEC)) for b in range(NBLK))

    nc = bacc.Bacc(None, target_bir_lowering=False, debug=False, num_swdge_queues=4)
    f32 = mybir.dt.float32
    bf16 = mybir.dt.bfloat16
    fp8 = mybir.dt.float8e4
    i16 = mybir.dt.int16

    T1 = nc.dram_tensor("T1", [NTOT, F_IN], bf16, kind="ExternalInput")
    IDX = nc.dram_tensor("IDX", [128, CIDX], i16, kind="ExternalInput")
    SVAL = nc.dram_tensor("SVAL", [128, TG, 128], fp8, kind="ExternalInput")
    DINV = nc.dram_tensor("DINV", [128, NG], f32, kind="ExternalInput")
    T1D = nc.dram_tensor("T1D", [128, NG * 128], bf16, kind="ExternalInput")
    W1T = nc.dram_tensor("W1T", [F_IN, HID], f32, kind="ExternalInput")
    B1T = nc.dram_tensor("B1T", [128, HID], f32, kind="ExternalInput")
    W2T = nc.dram_tensor("W2T", [HID, OUT_D], f32, kind="ExternalInput")
    B2T = nc.dram_tensor("B2T", [128, OUT_D], f32, kind="ExternalInput")
    OUTE = nc.dram_tensor("OUTE", [128, NG * OUT_D], f32, kind="ExternalOutput")

    t_local = nc.dram_tensor("t_local", [128, NG * F_IN], bf16)
    t_full = nc.dram_tensor("t_full", [NTOT, F_IN], bf16, addr_space="Shared")

    with tile.TileContext(nc) as tc:
        with (
            tc.tile_pool(name="resident", bufs=1) as rpool,
            tc.tile_pool(name="msg", bufs=MSGBUF) as mpool,
            tc.tile_pool(name="sv", bufs=2) as spool,
            tc.tile_pool(name="post", bufs=3) as ppool,
            tc.tile_pool(name="tiny", bufs=3) as ypool,
            tc.tile_pool(name="acc", bufs=2, space="PSUM") as apool,
            tc.tile_pool(name="tp", bufs=2, space="PSUM") as tpool,
        ):
            idx_t = rpool.tile([128, CIDX], i16)
            nc.sync.dma_start(idx_t[:], IDX[:])
            dinv_t = rpool.tile([128, NG], f32)
            nc.sync.dma_start(dinv_t[:], DINV[:])
            w1_t = rpool.tile([F_IN, HID], f32)
            nc.sync.dma_start(w1_t[:], W1T[:])
            b1_t = rpool.tile([128, HID], f32)
            nc.sync.dma_start(b1_t[:], B1T[:])
            w2_t = rpool.tile([HID, OUT_D], f32)
            nc.sync.dma_start(w2_t[:], W2T[:])
            b2_t = rpool.tile([128, OUT_D], f32)
            nc.sync.dma_start(b2_t[:], B2T[:])
            ident = rpool.tile([128, 128], bf16)
            make_identity(nc, ident[:])

            ht_t = rpool.tile([HID, NG * 128], bf16)
            oute_t = rpool.tile([128, NG * OUT_D], f32)

            dma_sems = [nc.alloc_semaphore(f"gdma{q}") for q in range(4)]

            def issue_gathers(table, prep_n=0):
                msg_tiles = {}
                entries = []
                kmax = max(len(cs) for cs in chunks)
                for k in range(kmax):
                    for s in range(NSEC):
                        if k < len(chunks[s]):
                            entries.append((s, k, chunks[s][k]))
                for i, (s, k, csz) in enumerate(entries):
                    qn = i % 4
                    ng = csz // 128
                    msg = mpool.tile([128, CHUNK // 128, F_IN], bf16, tag="msg")
                    kw = {}
                    if i < prep_n:
                        kw = dict(prepare_only=True, sem=dma_sems[qn])
                    nc.gpsimd.dma_gather(
                        msg[:, :ng, :],
                        table[s * SEC:(s + 1) * SEC, :],
                        idx_t[:, sec_coff[s] + k * (CHUNK // 16):
                              sec_coff[s] + k * (CHUNK // 16) + csz // 16],
                        csz,
                        csz,
                        F_IN,
                        single_packet=True,
                        queue_num=qn,
                        **kw,
                    )
                    if i == prep_n - 1:
                        for q in range(4):
                            nc.gpsimd.trigger_dma(count=None, queue_num=q)
                    msg_tiles[(s, k)] = msg
                return msg_tiles

            def scatter_block(b, msg_tiles, macc, mwid):
                gblk = sum(len(groups[b][s]) for s in range(NSEC))
                g0 = sum(sum(len(groups[bb][s]) for s in range(NSEC))
                         for bb in range(b))
                s_t = spool.tile([128, GBLK_MAX, 128], fp8, tag="sval")
                nc.sync.dma_start(s_t[:, :gblk, :], SVAL[:, g0:g0 + gblk, :])
                acc = apool.tile([macc, BCOLS], f32, tag=f"acc{macc}")
                mm = 0
                for s in range(NSEC):
                    for (tk, jj, base) in groups[b][s]:
                        nc.tensor.matmul(
                            acc[:, base:base + 128],
                            msg_tiles[(s, tk)][:, jj, :mwid],
                            s_t[:, mm, :],
                            start=(mm == 0),
                            stop=(mm == gblk - 1),
                        )
                        mm += 1
                return acc

            # ---------------- layer 1 ----------------
            msg_tiles = issue_gathers(T1)
            for b in range(NBLK):
                blo = b * BCOLS
                blen = min(BCOLS, NPC - blo)
                acc = scatter_block(b, msg_tiles, 128, 128)
                gcols = _round128(blen)
                t1d_b = ypool.tile([128, BCOLS], bf16, tag="t1d")
                nc.sync.dma_start(t1d_b[:, :gcols], T1D[:, blo:blo + gcols])
                hb = ppool.tile([128, BCOLS], f32, tag="hb1")
                nc.vector.tensor_tensor(
                    out=hb[:, :blen], in0=acc[:, :blen],
                    in1=t1d_b[:, :blen], op=mybir.AluOpType.add)
                if blen < BCOLS:
                    nc.vector.memset(hb[:, blen:_round128(blen)], 0.0)
                tloc_b = ppool.tile([128, BCOLS], bf16, tag="tloc")
                nc.vector.memset(tloc_b[:], 0.0)
                for g in range(blo // 128, (blo + blen + 127) // 128):
                    go = g * 128 - blo
                    pt = tpool.tile([128, 128], f32, tag="t")
                    nc.tensor.matmul(
                        pt[:, :HID], hb[:, go:go + 128], w1_t[:],
                        start=True, stop=True)
                    tmp = ypool.tile([128, HID], f32, tag="tmp")
                    nc.vector.tensor_scalar(
                        out=tmp[:], in0=pt[:, :HID], scalar1=dinv_t[:, g:g + 1],
                        scalar2=None, op0=mybir.AluOpType.mult)
                    nc.vector.tensor_tensor(
                        out=tmp[:], in0=tmp[:], in1=b1_t[:],
                        op=mybir.AluOpType.add)
                    # relu + src-side dinv pre-scale, written into the table
                    plim = 84 if g == NG - 1 else 128
                    nc.vector.tensor_scalar(
                        out=tloc_b[:plim, go:go + HID],
                        in0=tmp[:plim, :], scalar1=0.0,
                        scalar2=dinv_t[:plim, g:g + 1],
                        op0=mybir.AluOpType.max, op1=mybir.AluOpType.mult)
                    # transposed copy for the layer-2 diagonal term
                    pt2 = tpool.tile([128, 128], bf16, tag="tb")
                    nc.tensor.transpose(
                        pt2[:HID, :], tloc_b[:, go:go + HID],
                        ident[:])
                    nc.vector.tensor_copy(
                        ht_t[:, g * 128:(g + 1) * 128], pt2[:HID, :])
                nc.sync.dma_start(
                    t_local[:, blo:blo + gcols], tloc_b[:, :gcols])

            nc.gpsimd.collective_compute(
                "AllGather",
                mybir.AluOpType.bypass,
                replica_groups=[list(range(NCORES))],
                ins=[t_local[:]],
                outs=[t_full[:]],
            )

            # ---------------- layer 2 ----------------
            msg_tiles = issue_gathers(t_full)
            for b in range(NBLK):
                blo = b * BCOLS
                blen = min(BCOLS, NPC - blo)
                acc = scatter_block(b, msg_tiles, HID, HID)
                hb = ppool.tile([HID, BCOLS], f32, tag="hb2")
                nc.vector.tensor_tensor(
                    out=hb[:, :blen], in0=acc[:, :blen],
                    in1=ht_t[:, blo:blo + blen], op=mybir.AluOpType.add)
                if blen < BCOLS:
                    nc.vector.memset(hb[:, blen:_round128(blen)], 0.0)
                for g in range(blo // 128, (blo + blen + 127) // 128):
                    go = g * 128 - blo
                    pt = tpool.tile([128, 128], f32, tag="t")
                    nc.tensor.matmul(
                        pt[:, :OUT_D], hb[:, go:go + 128], w2_t[:],
                        start=True, stop=True)
                    ot = ypool.tile([128, OUT_D], f32, tag="ot")
                    nc.vector.tensor_scalar(
                        out=ot[:], in0=pt[:, :OUT_D], scalar1=dinv_t[:, g:g + 1],
                        scalar2=None, op0=mybir.AluOpType.mult)
                    nc.vector.tensor_tensor(
                        out=oute_t[:, g * OUT_D:(g + 1) * OUT_D],
                        in0=ot[:], in1=b2_t[:], op=mybir.AluOpType.add)

            nc.sync.dma_start(OUTE[:], oute_t[:])

    nc.compile()
    return nc


# ============================ entry point ============================

def prepare(x, edge_index, W1, b1, W2, b2):
    x = np.asarray(x, dtype=np.float32)
    W1 = np.asarray(W1, dtype=np.float32)
    b1 = np.asarray(b1, dtype=np.float32)
    W2 = np.asarray(W2, dtype=np.float32)
    b2 = np.asarray(b2, dtype=np.float32)

    sched, idx_all, sval_all, dinv_gt, dinv = _host_prep(edge_index)

    key = ("v2", CHUNK, sched["CIDX"], sched["TG"])
    if key in _CACHE:
        nc = _CACHE[key]
    else:
        nc = build_program(sched)
        _CACHE[key] = nc

    xs = x * dinv[:, None]
    T1 = np.zeros((NTOT, F_IN), dtype=ml_dtypes.bfloat16)
    n = np.arange(N)
    sl = n % NPC
    srow = (n // NPC) * SH + (sl % 128) * NG + (sl // 128)
    T1[srow] = xs.astype(ml_dtypes.bfloat16)

    b1_tile = np.tile(b1[None, :], (128, 1)).astype(np.float32)
    b2_tile = np.tile(b2[None, :], (128, 1)).astype(np.float32)

    in_maps = []
    for c in range(NCORES):
        t1d = np.zeros((128, NG * 128), dtype=ml_dtypes.bfloat16)
        xsl = np.zeros((SH, F_IN), dtype=np.float32)
        xsl[:NPC] = xs[c * NPC:(c + 1) * NPC]
        # t1d[f, l] = xs[c*NPC + l][f]
        t1d[:, :] = xsl.T.astype(ml_dtypes.bfloat16)
        in_maps.append({
            "T1": T1,
            "IDX": np.ascontiguousarray(idx_all[c]),
            "SVAL": np.ascontiguousarray(sval_all[c]),
            "DINV": np.ascontiguousarray(dinv_gt[c]),
            "T1D": t1d,
            "W1T": W1,
            "B1T": b1_tile,
            "W2T": W2,
            "B2T": b2_tile,
        })
    return nc, in_maps


def assemble_output(results):
    out = np.empty((N, OUT_D), dtype=np.float32)
    for c in range(NCORES):
        oc = np.asarray(results[c]["OUTE"]).reshape(128, NG, OUT_D)
        out[c * NPC:(c + 1) * NPC] = (
            oc.transpose(1, 0, 2).reshape(SH, OUT_D)[:NPC])
    return out


def kernel(x, edge_index, W1, b1, W2, b2):
    from concourse.bass_utils import run_bass_kernel_spmd

    nc, in_maps = prepare(x, edge_index, W1, b1, W2, b2)
    r = run_bass_kernel_spmd(nc, in_maps, core_ids=list(range(NCORES)))
    return assemble_output(r.results)


# revision 23
# speedup vs baseline: 1.7905x; 1.0296x over previous
"""Trainium2 Bass kernel for 2-layer GCN (nn_GCN_22866405884174).

v2 strategy (8 NeuronCores, dst-node sharding):
  out = A @ relu((A @ x) @ W1 + b1) @ W2 + b2   with A = D^-1/2 (Adj+I) D^-1/2

  Each layer = per-edge gather (SWDGE dma_gather) of pre-scaled source rows,
  one-hot fp8 scatter matmul into a PSUM block [*, 512 dst], then a small
  dense transform per 128-dst group.

  v2 changes vs v1 (trace-driven):
  - Self-loops removed from the edge stream; the diagonal term is added
    densely from transposed local tables (T1D input / on-device PE
    transpose of the hidden layer).
  - Exact per-core edge streams: slots per (sec, block) cell padded only to
    the max across cores rounded to 128 (quantile-spread groups, per-group
    column bases chosen host-side) -- kills the per-window 16-align+128-min
    padding of v1 (~-20% descriptors).
  - SVAL stored pre-transposed [128, TG, 128] so each per-block load is one
    contiguous 2D DMA (v1's rearrange cost ~1.5ms of HWDGE gen on Sync).
  - Tables stored partition-major (row = c*SH + p*NG + g) so t_local and
    OUTE are written with a single contiguous DMA each instead of 98 2D
    DMAs per layer.
  - Layer-2 scatter matmuls use M=64 (only the real hidden columns).
  - Larger gather chunks to amortize the ~1us fixed SWDGE cost per
    instruction (descriptor generation on the Q7 is the serial bottleneck).
"""

import numpy as np
import ml_dtypes

# ---------------- problem constants (hardcoded per contract) ----------------
N = 100000
E = 1600000
F_IN = 128
HID = 64
OUT_D = 10

NCORES = 8
NPC = N // NCORES           # 12500 nodes per core
NG = 98                     # node groups of 128 per shard
SH = NG * 128               # 12544 padded shard rows
NTOT = SH * NCORES          # 100352
SEC = 25088                 # table section rows (2 shards, < int16 range)
NSEC = 4
BCOLS = 512                 # psum block width (dst cols)
NBLK = (NPC + BCOLS - 1) // BCOLS   # 25 (last block = 212 dst)
CHUNK = 1024                # gather chunk (tunable; single_packet limit 1024)
MSGBUF = 16                 # in-flight gather chunk tiles
PREPN = 0                   # prep/trigger overlap disabled (breaks consumer waits)
SENT_LOCAL = 84 * NG + 97   # zero pad row (p=84, g=97 -> node 12500), per section

_CACHE = {}


# ============================ host preprocessing ============================

def _round128(v):
    return ((int(v) + 127) // 128) * 128


def _host_prep(edge_index):
    src = np.asarray(edge_index[0]).astype(np.int64)
    dst = np.asarray(edge_index[1]).astype(np.int64)
    deg = (np.bincount(dst, minlength=N) + 1.0).astype(np.float32)  # + self loop
    dinv = (1.0 / np.sqrt(deg)).astype(np.float32)

    core = dst // NPC
    dloc = dst % NPC
    blk = dloc // BCOLS
    sl = src % NPC
    srow = (src // NPC) * SH + (sl % 128) * NG + (sl // 128)
    sec = srow // SEC

    order = np.lexsort((dloc, blk, sec, core))
    srow_s = srow[order]
    dloc_s = dloc[order]
    cell = ((core * NSEC + sec) * NBLK + blk)[order]
    starts = np.searchsorted(cell, np.arange(NCORES * NSEC * NBLK + 1))
    cnt = np.diff(starts).reshape(NCORES, NSEC, NBLK)

    # --- per (sec, blk): run size + group column bases (shared across cores) ---
    run = np.zeros((NSEC, NBLK), np.int64)
    bases = [[None] * NBLK for _ in range(NSEC)]
    for s in range(NSEC):
        for b in range(NBLK):
            r = max(_round128(cnt[:, s, b].max()), 128)
            while True:
                ngrp = r // 128
                lo = np.full(ngrp, BCOLS, np.int64)
                hi = np.full(ngrp, -1, np.int64)
                for c in range(NCORES):
                    m = cnt[c, s, b]
                    if m == 0:
                        continue
                    a = starts[(c * NSEC + s) * NBLK + b]
                    dc = dloc_s[a:a + m] - b * BCOLS
                    tj = (np.arange(ngrp + 1) * m) // ngrp
                    for j in range(ngrp):
                        if tj[j + 1] > tj[j]:
                            lo[j] = min(lo[j], dc[tj[j]])
                            hi[j] = max(hi[j], dc[tj[j + 1] - 1])
                bj = np.clip(lo, 0, BCOLS - 128)
                if np.all(hi < bj + 128):
                    bases[s][b] = bj
                    run[s, b] = r
                    break
                r += 128
    soff = np.zeros((NSEC, NBLK), np.int64)
    sec_len = np.zeros(NSEC, np.int64)
    for s in range(NSEC):
        soff[s] = np.concatenate([[0], np.cumsum(run[s])[:-1]])
        sec_len[s] = run[s].sum()

    sec_coff = np.concatenate([[0], np.cumsum(sec_len // 16)[:-1]]).astype(np.int64)
    CIDX = int((sec_len // 16).sum())

    chunks = []
    for s in range(NSEC):
        rem = int(sec_len[s])
        cs = []
        while rem > 0:
            cs.append(min(CHUNK, rem))
            rem -= cs[-1]
        chunks.append(cs)

    # program group order: block-major (b, s, j)
    gi0 = np.zeros((NBLK, NSEC), np.int64)
    TG = 0
    groups = [[None] * NSEC for _ in range(NBLK)]
    for b in range(NBLK):
        for s in range(NSEC):
            gi0[b, s] = TG
            ng = int(run[s, b]) // 128
            gl = []
            for j in range(ng):
                gslot = int(soff[s, b]) + j * 128
                gl.append((gslot // CHUNK, (gslot % CHUNK) // 128,
                           int(bases[s][b][j])))
            groups[b][s] = gl
            TG += ng

    # --- per-core streams + S values ---
    idx_all = np.zeros((NCORES, 128, CIDX), dtype=np.int16)
    sval_all = np.zeros((NCORES, 128, TG, 128), dtype=ml_dtypes.float8_e4m3)
    dinv_gt = np.zeros((NCORES, 128, NG), dtype=np.float32)

    for c in range(NCORES):
        p_l, g_l, w_l = [], [], []
        for s in range(NSEC):
            stream = np.full(int(sec_len[s]), SENT_LOCAL, dtype=np.int16)
            for b in range(NBLK):
                m = cnt[c, s, b]
                if m == 0:
                    continue
                a = starts[(c * NSEC + s) * NBLK + b]
                sr = srow_s[a:a + m] - s * SEC
                dc = dloc_s[a:a + m] - b * BCOLS
                ngrp = int(run[s, b]) // 128
                tj = (np.arange(ngrp + 1) * m) // ngrp
                t = np.arange(m)
                j = np.searchsorted(tj, t, side="right") - 1
                p = t - tj[j]
                pos = int(soff[s, b]) + j * 128 + p
                stream[pos] = sr.astype(np.int16)
                w = dc - bases[s][b][j]
                assert w.min() >= 0 and w.max() < 128
                p_l.append(p)
                g_l.append(gi0[b, s] + j)
                w_l.append(w)
            ncol = int(sec_len[s]) // 16
            idx_all[c, :, sec_coff[s]:sec_coff[s] + ncol] = np.tile(
                stream.reshape(ncol, 16).T, (8, 1))
        sval_all[c, np.concatenate(p_l), np.concatenate(g_l),
                 np.concatenate(w_l)] = ml_dtypes.float8_e4m3(1.0)

        dpad = np.zeros(SH, dtype=np.float32)
        dpad[:NPC] = dinv[c * NPC:(c + 1) * NPC]
        dinv_gt[c] = dpad.reshape(NG, 128).T

    sched = {
        "groups": groups, "chunks": chunks, "sec_coff": sec_coff,
        "sec_len": sec_len, "CIDX": CIDX, "TG": TG,
    }
    return sched, idx_all, sval_all, dinv_gt, dinv


# ============================ device program ============================

def build_program(sched):
    import concourse.bass as bass  # noqa: F401
    import concourse.bacc as bacc
    import concourse.tile as tile
    import concourse.mybir as mybir
    from concourse.masks import make_identity
    from concourse.instruction_name_ordered_set import InstructionNameOrderedSet

    CIDX = sched["CIDX"]
    TG = sched["TG"]
    groups = sched["groups"]
    chunks = sched["chunks"]
    sec_coff = sched["sec_coff"]
    GBLK_MAX = max(sum(len(groups[b][s]) for s in range(NSEC)) for b in range(NBLK))

    nc = bacc.Bacc(None, target_bir_lowering=False, debug=False, num_swdge_queues=4)
    f32 = mybir.dt.float32
    bf16 = mybir.dt.bfloat16
    fp8 = mybir.dt.float8e4
    i16 = mybir.dt.int16

    T1 = nc.dram_tensor("T1", [NTOT, F_IN], bf16, kind="ExternalInput")
    IDX = nc.dram_tensor("IDX", [128, CIDX], i16, kind="ExternalInput")
    SVAL = nc.dram_tensor("SVAL", [128, TG, 128], fp8, kind="ExternalInput")
    DINV = nc.dram_tensor("DINV", [128, NG], f32, kind="ExternalInput")
    T1D = nc.dram_tensor("T1D", [128, NG * 128], bf16, kind="ExternalInput")
    W1T = nc.dram_tensor("W1T", [F_IN, HID], f32, kind="ExternalInput")
    B1T = nc.dram_tensor("B1T", [128, HID], f32, kind="ExternalInput")
    W2T = nc.dram_tensor("W2T", [HID, OUT_D], f32, kind="ExternalInput")
    B2T = nc.dram_tensor("B2T", [128, OUT_D], f32, kind="ExternalInput")
    OUTE = nc.dram_tensor("OUTE", [128, NG * OUT_D], f32, kind="ExternalOutput")

    t_local = nc.dram_tensor("t_local", [128, NG * F_IN], bf16)
    t_full = nc.dram_tensor("t_full", [NTOT, F_IN], bf16, addr_space="Shared")

    with tile.TileContext(nc) as tc:
        with (
            tc.tile_pool(name="resident", bufs=1) as rpool,
            tc.tile_pool(name="msg", bufs=MSGBUF) as mpool,
            tc.tile_pool(name="sv", bufs=2) as spool,
            tc.tile_pool(name="post", bufs=3) as ppool,
            tc.tile_pool(name="tiny", bufs=3) as ypool,
            tc.tile_pool(name="acc", bufs=2, space="PSUM") as apool,
            tc.tile_pool(name="tp", bufs=2, space="PSUM") as tpool,
        ):
            idx_t = rpool.tile([128, CIDX], i16)
            nc.sync.dma_start(idx_t[:], IDX[:])
            dinv_t = rpool.tile([128, NG], f32)
            nc.sync.dma_start(dinv_t[:], DINV[:])
            w1_t = rpool.tile([F_IN, HID], f32)
            nc.sync.dma_start(w1_t[:], W1T[:])
            b1_t = rpool.tile([128, HID], f32)
            nc.sync.dma_start(b1_t[:], B1T[:])
            w2_t = rpool.tile([HID, OUT_D], f32)
            nc.sync.dma_start(w2_t[:], W2T[:])
            b2_t = rpool.tile([128, OUT_D], f32)
            nc.sync.dma_start(b2_t[:], B2T[:])
            ident = rpool.tile([128, 128], bf16)
            make_identity(nc, ident[:])

            ht_t = rpool.tile([HID, NG * 128], bf16)
            oute_t = rpool.tile([128, NG * OUT_D], f32)

            dma_sems = [nc.alloc_semaphore(f"gdma{q}") for q in range(4)]

            def issue_gathers(table, prep_n=0, gate_name=None, snif=None):
                msg_tiles = {}
                entries = []
                kmax = max(len(cs) for cs in chunks)
                for k in range(kmax):
                    for s in range(NSEC):
                        if k < len(chunks[s]):
                            entries.append((s, k, chunks[s][k]))
                for i, (s, k, csz) in enumerate(entries):
                    qn = i % 4
                    ng = csz // 128
                    msg = mpool.tile([128, CHUNK // 128, F_IN], bf16, tag="msg")
                    kw = {}
                    if i < prep_n:
                        kw = dict(prepare_only=True, sem=dma_sems[qn])
                    h = nc.gpsimd.dma_gather(
                        msg[:, :ng, :],
                        table[s * SEC:(s + 1) * SEC, :],
                        idx_t[:, sec_coff[s] + k * (CHUNK // 16):
                              sec_coff[s] + k * (CHUNK // 16) + csz // 16],
                        csz,
                        csz,
                        F_IN,
                        single_packet=True,
                        queue_num=qn,
                        **kw,
                    )
                    if i < prep_n and gate_name is not None:
                        # Desc-gen only reads idx; the table read happens at
                        # trigger time (gated via the sniffer WAW below), so
                        # demote the collective -> prep RAW edge to no-sync.
                        if h.ins.try_remove_dependency(gate_name):
                            ns = InstructionNameOrderedSet()
                            ns.add(gate_name)
                            h.ins.add_nosync_dependencies_from(ns)
                    if i == prep_n - 1:
                        for q in range(4):
                            nc.gpsimd.trigger_dma(
                                count=None, queue_num=q,
                                signals_writable=(
                                    [snif[:]] if snif is not None else ()),
                            )
                    msg_tiles[(s, k)] = msg
                return msg_tiles

            def scatter_block(b, msg_tiles, macc, mwid):
                gblk = sum(len(groups[b][s]) for s in range(NSEC))
                g0 = sum(sum(len(groups[bb][s]) for s in range(NSEC))
                         for bb in range(b))
                s_t = spool.tile([128, GBLK_MAX, 128], fp8, tag="sval")
                nc.sync.dma_start(s_t[:, :gblk, :], SVAL[:, g0:g0 + gblk, :])
                acc = apool.tile([macc, BCOLS], f32, tag=f"acc{macc}")
                mm = 0
                for s in range(NSEC):
                    for (tk, jj, base) in groups[b][s]:
                        nc.tensor.matmul(
                            acc[:, base:base + 128],
                            msg_tiles[(s, tk)][:, jj, :mwid],
                            s_t[:, mm, :],
                            start=(mm == 0),
                            stop=(mm == gblk - 1),
                        )
                        mm += 1
                return acc

            # ---------------- layer 1 ----------------
            msg_tiles = issue_gathers(T1)
            for b in range(NBLK):
                blo = b * BCOLS
                blen = min(BCOLS, NPC - blo)
                acc = scatter_block(b, msg_tiles, 128, 128)
                gcols = _round128(blen)
                t1d_b = ypool.tile([128, BCOLS], bf16, tag="t1d")
                nc.sync.dma_start(t1d_b[:, :gcols], T1D[:, blo:blo + gcols])
                hb = ppool.tile([128, BCOLS], f32, tag="hb1")
                nc.vector.tensor_tensor(
                    out=hb[:, :blen], in0=acc[:, :blen],
                    in1=t1d_b[:, :blen], op=mybir.AluOpType.add)
                if blen < BCOLS:
                    nc.vector.memset(hb[:, blen:_round128(blen)], 0.0)
                tloc_b = ppool.tile([128, BCOLS], bf16, tag="tloc")
                nc.vector.memset(tloc_b[:], 0.0)
                for g in range(blo // 128, (blo + blen + 127) // 128):
                    go = g * 128 - blo
                    pt = tpool.tile([128, 128], f32, tag="t")
                    nc.tensor.matmul(
                        pt[:, :HID], hb[:, go:go + 128], w1_t[:],
                        start=True, stop=True)
                    tmp = ypool.tile([128, HID], f32, tag="tmp")
                    nc.vector.tensor_scalar(
                        out=tmp[:], in0=pt[:, :HID], scalar1=dinv_t[:, g:g + 1],
                        scalar2=None, op0=mybir.AluOpType.mult)
                    nc.vector.tensor_tensor(
                        out=tmp[:], in0=tmp[:], in1=b1_t[:],
                        op=mybir.AluOpType.add)
                    # relu + src-side dinv pre-scale, written into the table
                    plim = 84 if g == NG - 1 else 128
                    nc.vector.tensor_scalar(
                        out=tloc_b[:plim, go:go + HID],
                        in0=tmp[:plim, :], scalar1=0.0,
                        scalar2=dinv_t[:plim, g:g + 1],
                        op0=mybir.AluOpType.max, op1=mybir.AluOpType.mult)
                nc.sync.dma_start(
                    t_local[:, blo:blo + gcols], tloc_b[:, :gcols])

            cc = nc.gpsimd.collective_compute(
                "AllGather",
                mybir.AluOpType.bypass,
                replica_groups=[list(range(NCORES))],
                ins=[t_local[:]],
                outs=[t_full[:]],
            )
            # Build the transposed hidden table (layer-2 diagonal term) from
            # t_local during the collective -- PE/DVE are idle in that window.
            tl_t = rpool.tile([128, NG * F_IN], bf16)
            nc.sync.dma_start(tl_t[:], t_local[:])
            for g in range(NG):
                pt2 = tpool.tile([128, 128], bf16, tag="tb")
                nc.tensor.transpose(
                    pt2[:HID, :], tl_t[:, g * F_IN:g * F_IN + HID], ident[:])
                nc.vector.tensor_copy(
                    ht_t[:, g * 128:(g + 1) * 128], pt2[:HID, :])
            snif = None

            # ---------------- layer 2 ----------------
            msg_tiles = issue_gathers(t_full, prep_n=PREPN,
                                      gate_name=cc.ins.name, snif=snif)
            for b in range(NBLK):
                blo = b * BCOLS
                blen = min(BCOLS, NPC - blo)
                acc = scatter_block(b, msg_tiles, HID, HID)
                hb = ppool.tile([HID, BCOLS], f32, tag="hb2")
                nc.vector.tensor_tensor(
                    out=hb[:, :blen], in0=acc[:, :blen],
                    in1=ht_t[:, blo:blo + blen], op=mybir.AluOpType.add)
                if blen < BCOLS:
                    nc.vector.memset(hb[:, blen:_round128(blen)], 0.0)
                for g in range(blo // 128, (blo + blen + 127) // 128):
                    go = g * 128 - blo
                    pt = tpool.tile([128, 128], f32, tag="t")
                    nc.tensor.matmul(
                        pt[:, :OUT_D], hb[:, go:go + 128], w2_t[:],
                        start=True, stop=True)
                    ot = ypool.tile([128, OUT_D], f32, tag="ot")
                    nc.vector.tensor_scalar(
                        out=ot[:], in0=pt[:, :OUT_D], scalar1=dinv_t[:, g:g + 1],
                        scalar2=None, op0=mybir.AluOpType.mult)
                    nc.vector.tensor_tensor(
                        out=oute_t[:, g * OUT_D:(g + 1) * OUT_D],
                        in0=ot[:], in1=b2_t[:], op=mybir.AluOpType.add)

            nc.sync.dma_start(OUTE[:], oute_t[:])

    nc.compile()
    return nc


# ============================ entry point ============================

def prepare(x, edge_index, W1, b1, W2, b2):
    x = np.asarray(x, dtype=np.float32)
    W1 = np.asarray(W1, dtype=np.float32)
    b1 = np.asarray(b1, dtype=np.float32)
    W2 = np.asarray(W2, dtype=np.float32)
    b2 = np.asarray(b2, dtype=np.float32)

    sched, idx_all, sval_all, dinv_gt, dinv = _host_prep(edge_index)

    key = ("v2", CHUNK, sched["CIDX"], sched["TG"])
    if key in _CACHE:
        nc = _CACHE[key]
    else:
        nc = build_program(sched)
        _CACHE[key] = nc

    xs = x * dinv[:, None]
    T1 = np.zeros((NTOT, F_IN), dtype=ml_dtypes.bfloat16)
    n = np.arange(N)
    sl = n % NPC
    srow = (n // NPC) * SH + (sl % 128) * NG + (sl // 128)
    T1[srow] = xs.astype(ml_dtypes.bfloat16)

    b1_tile = np.tile(b1[None, :], (128, 1)).astype(np.float32)
    b2_tile = np.tile(b2[None, :], (128, 1)).astype(np.float32)

    in_maps = []
    for c in range(NCORES):
        t1d = np.zeros((128, NG * 128), dtype=ml_dtypes.bfloat16)
        xsl = np.zeros((SH, F_IN), dtype=np.float32)
        xsl[:NPC] = xs[c * NPC:(c + 1) * NPC]
        # t1d[f, l] = xs[c*NPC + l][f]
        t1d[:, :] = xsl.T.astype(ml_dtypes.bfloat16)
        in_maps.append({
            "T1": T1,
            "IDX": np.ascontiguousarray(idx_all[c]),
            "SVAL": np.ascontiguousarray(sval_all[c]),
            "DINV": np.ascontiguousarray(dinv_gt[c]),
            "T1D": t1d,
            "W1T": W1,
            "B1T": b1_tile,
            "W2T": W2,
            "B2T": b2_tile,
        })
    return nc, in_maps


def assemble_output(results):
    out = np.empty((N, OUT_D), dtype=np.float32)
    for c in range(NCORES):
        oc = np.asarray(results[c]["OUTE"]).reshape(128, NG, OUT_D)
        out[c * NPC:(c + 1) * NPC] = (
            oc.transpose(1, 0, 2).reshape(SH, OUT_D)[:NPC])
    return out


def kernel(x, edge_index, W1, b1, W2, b2):
    from concourse.bass_utils import run_bass_kernel_spmd

    nc, in_maps = prepare(x, edge_index, W1, b1, W2, b2)
    r = run_bass_kernel_spmd(nc, in_maps, core_ids=list(range(NCORES)))
    return assemble_output(r.results)


# revision 30
# speedup vs baseline: 1.7979x; 1.0041x over previous
"""Trainium2 Bass kernel for 2-layer GCN (nn_GCN_22866405884174).

v2 strategy (8 NeuronCores, dst-node sharding):
  out = A @ relu((A @ x) @ W1 + b1) @ W2 + b2   with A = D^-1/2 (Adj+I) D^-1/2

  Each layer = per-edge gather (SWDGE dma_gather) of pre-scaled source rows,
  one-hot fp8 scatter matmul into a PSUM block [*, 512 dst], then a small
  dense transform per 128-dst group.

  v2 changes vs v1 (trace-driven):
  - Self-loops removed from the edge stream; the diagonal term is added
    densely from transposed local tables (T1D input / on-device PE
    transpose of the hidden layer).
  - Exact per-core edge streams: slots per (sec, block) cell padded only to
    the max across cores rounded to 128 (quantile-spread groups, per-group
    column bases chosen host-side) -- kills the per-window 16-align+128-min
    padding of v1 (~-20% descriptors).
  - SVAL stored pre-transposed [128, TG, 128] so each per-block load is one
    contiguous 2D DMA (v1's rearrange cost ~1.5ms of HWDGE gen on Sync).
  - Tables stored partition-major (row = c*SH + p*NG + g) so t_local and
    OUTE are written with a single contiguous DMA each instead of 98 2D
    DMAs per layer.
  - Layer-2 scatter matmuls use M=64 (only the real hidden columns).
  - Larger gather chunks to amortize the ~1us fixed SWDGE cost per
    instruction (descriptor generation on the Q7 is the serial bottleneck).
"""

import numpy as np
import ml_dtypes

# ---------------- problem constants (hardcoded per contract) ----------------
N = 100000
E = 1600000
F_IN = 128
HID = 64
OUT_D = 10

NCORES = 8
NPC = N // NCORES           # 12500 nodes per core
NG = 98                     # node groups of 128 per shard
SH = NG * 128               # 12544 padded shard rows
NTOT = SH * NCORES          # 100352
SEC = 25088                 # table section rows (2 shards, < int16 range)
NSEC = 4
BCOLS = 512                 # psum block width (dst cols)
NBLK = (NPC + BCOLS - 1) // BCOLS   # 25 (last block = 212 dst)
CHUNK = 1024                # gather chunk (tunable; single_packet limit 1024)
MSGBUF = 28                 # in-flight gather chunk tiles
PREPN = 0                   # prep/trigger overlap disabled (breaks consumer waits)
SENT_LOCAL = 84 * NG + 97   # zero pad row (p=84, g=97 -> node 12500), per section

_CACHE = {}


# ============================ host preprocessing ============================

def _round128(v):
    return ((int(v) + 127) // 128) * 128


def _host_prep(edge_index):
    src = np.asarray(edge_index[0]).astype(np.int64)
    dst = np.asarray(edge_index[1]).astype(np.int64)
    deg = (np.bincount(dst, minlength=N) + 1.0).astype(np.float32)  # + self loop
    dinv = (1.0 / np.sqrt(deg)).astype(np.float32)

    core = dst // NPC
    dloc = dst % NPC
    blk = dloc // BCOLS
    sl = src % NPC
    srow = (src // NPC) * SH + (sl % 128) * NG + (sl // 128)
    sec = srow // SEC

    order = np.lexsort((dloc, blk, sec, core))
    srow_s = srow[order]
    dloc_s = dloc[order]
    cell = ((core * NSEC + sec) * NBLK + blk)[order]
    starts = np.searchsorted(cell, np.arange(NCORES * NSEC * NBLK + 1))
    cnt = np.diff(starts).reshape(NCORES, NSEC, NBLK)

    # --- per (sec, blk): run size + group column bases (shared across cores) ---
    run = np.zeros((NSEC, NBLK), np.int64)
    bases = [[None] * NBLK for _ in range(NSEC)]
    for s in range(NSEC):
        for b in range(NBLK):
            r = max(_round128(cnt[:, s, b].max()), 128)
            while True:
                ngrp = r // 128
                lo = np.full(ngrp, BCOLS, np.int64)
                hi = np.full(ngrp, -1, np.int64)
                for c in range(NCORES):
                    m = cnt[c, s, b]
                    if m == 0:
                        continue
                    a = starts[(c * NSEC + s) * NBLK + b]
                    dc = dloc_s[a:a + m] - b * BCOLS
                    tj = (np.arange(ngrp + 1) * m) // ngrp
                    for j in range(ngrp):
                        if tj[j + 1] > tj[j]:
                            lo[j] = min(lo[j], dc[tj[j]])
                            hi[j] = max(hi[j], dc[tj[j + 1] - 1])
                bj = np.clip(lo, 0, BCOLS - 128)
                if np.all(hi < bj + 128):
                    bases[s][b] = bj
                    run[s, b] = r
                    break
                r += 128
    soff = np.zeros((NSEC, NBLK), np.int64)
    sec_len = np.zeros(NSEC, np.int64)
    for s in range(NSEC):
        soff[s] = np.concatenate([[0], np.cumsum(run[s])[:-1]])
        sec_len[s] = run[s].sum()

    sec_coff = np.concatenate([[0], np.cumsum(sec_len // 16)[:-1]]).astype(np.int64)
    CIDX = int((sec_len // 16).sum())

    chunks = []
    for s in range(NSEC):
        rem = int(sec_len[s])
        cs = []
        while rem > 0:
            cs.append(min(CHUNK, rem))
            rem -= cs[-1]
        chunks.append(cs)

    # program group order: block-major (b, s, j)
    gi0 = np.zeros((NBLK, NSEC), np.int64)
    TG = 0
    groups = [[None] * NSEC for _ in range(NBLK)]
    for b in range(NBLK):
        for s in range(NSEC):
            gi0[b, s] = TG
            ng = int(run[s, b]) // 128
            gl = []
            for j in range(ng):
                gslot = int(soff[s, b]) + j * 128
                gl.append((gslot // CHUNK, (gslot % CHUNK) // 128,
                           int(bases[s][b][j])))
            groups[b][s] = gl
            TG += ng

    # --- per-core streams + S values ---
    idx_all = np.zeros((NCORES, 128, CIDX), dtype=np.int16)
    sval_all = np.zeros((NCORES, 128, TG, 128), dtype=ml_dtypes.float8_e4m3)
    dinv_gt = np.zeros((NCORES, 128, NG), dtype=np.float32)

    for c in range(NCORES):
        p_l, g_l, w_l = [], [], []
        for s in range(NSEC):
            stream = np.full(int(sec_len[s]), SENT_LOCAL, dtype=np.int16)
            for b in range(NBLK):
                m = cnt[c, s, b]
                if m == 0:
                    continue
                a = starts[(c * NSEC + s) * NBLK + b]
                sr = srow_s[a:a + m] - s * SEC
                dc = dloc_s[a:a + m] - b * BCOLS
                ngrp = int(run[s, b]) // 128
                tj = (np.arange(ngrp + 1) * m) // ngrp
                t = np.arange(m)
                j = np.searchsorted(tj, t, side="right") - 1
                p = t - tj[j]
                pos = int(soff[s, b]) + j * 128 + p
                stream[pos] = sr.astype(np.int16)
                w = dc - bases[s][b][j]
                assert w.min() >= 0 and w.max() < 128
                p_l.append(p)
                g_l.append(gi0[b, s] + j)
                w_l.append(w)
            ncol = int(sec_len[s]) // 16
            idx_all[c, :, sec_coff[s]:sec_coff[s] + ncol] = np.tile(
                stream.reshape(ncol, 16).T, (8, 1))
        sval_all[c, np.concatenate(p_l), np.concatenate(g_l),
                 np.concatenate(w_l)] = ml_dtypes.float8_e4m3(1.0)

        dpad = np.zeros(SH, dtype=np.float32)
        dpad[:NPC] = dinv[c * NPC:(c + 1) * NPC]
        dinv_gt[c] = dpad.reshape(NG, 128).T

    sched = {
        "groups": groups, "chunks": chunks, "sec_coff": sec_coff,
        "sec_len": sec_len, "CIDX": CIDX, "TG": TG,
    }
    return sched, idx_all, sval_all, dinv_gt, dinv


# ============================ device program ============================

def build_program(sched):
    import concourse.bass as bass  # noqa: F401
    import concourse.bacc as bacc
    import concourse.tile as tile
    import concourse.mybir as mybir
    from concourse.masks import make_identity
    from concourse.instruction_name_ordered_set import InstructionNameOrderedSet

    CIDX = sched["CIDX"]
    TG = sched["TG"]
    groups = sched["groups"]
    chunks = sched["chunks"]
    sec_coff = sched["sec_coff"]
    GBLK_MAX = max(sum(len(groups[b][s]) for s in range(NSEC)) for b in range(NBLK))

    nc = bacc.Bacc(None, target_bir_lowering=False, debug=False, num_swdge_queues=4)
    f32 = mybir.dt.float32
    bf16 = mybir.dt.bfloat16
    fp8 = mybir.dt.float8e4
    i16 = mybir.dt.int16

    T1 = nc.dram_tensor("T1", [NTOT, F_IN], bf16, kind="ExternalInput")
    IDX = nc.dram_tensor("IDX", [128, CIDX], i16, kind="ExternalInput")
    SVAL = nc.dram_tensor("SVAL", [128, TG, 128], fp8, kind="ExternalInput")
    DINV = nc.dram_tensor("DINV", [128, NG], f32, kind="ExternalInput")
    T1D = nc.dram_tensor("T1D", [128, NG * 128], bf16, kind="ExternalInput")
    W1T = nc.dram_tensor("W1T", [F_IN, HID], f32, kind="ExternalInput")
    B1T = nc.dram_tensor("B1T", [128, HID], f32, kind="ExternalInput")
    W2T = nc.dram_tensor("W2T", [HID, OUT_D], f32, kind="ExternalInput")
    B2T = nc.dram_tensor("B2T", [128, OUT_D], f32, kind="ExternalInput")
    OUTE = nc.dram_tensor("OUTE", [128, NG * OUT_D], f32, kind="ExternalOutput")

    t_local = nc.dram_tensor("t_local", [128, NG * F_IN], bf16)
    t_full = nc.dram_tensor("t_full", [NTOT, F_IN], bf16, addr_space="Shared")

    with tile.TileContext(nc) as tc:
        with (
            tc.tile_pool(name="resident", bufs=1) as rpool,
            tc.tile_pool(name="msg", bufs=MSGBUF) as mpool,
            tc.tile_pool(name="sv", bufs=3) as spool,
            tc.tile_pool(name="post", bufs=3) as ppool,
            tc.tile_pool(name="tiny", bufs=3) as ypool,
            tc.tile_pool(name="acc", bufs=3, space="PSUM") as apool,
            tc.tile_pool(name="tp", bufs=2, space="PSUM") as tpool,
        ):
            idx_t = rpool.tile([128, CIDX], i16)
            for s in range(NSEC):
                c0 = int(sec_coff[s])
                c1 = int(sec_coff[s + 1]) if s + 1 < NSEC else CIDX
                nc.sync.dma_start(idx_t[:, c0:c1], IDX[:, c0:c1])
            dinv_t = rpool.tile([128, NG], f32)
            nc.sync.dma_start(dinv_t[:], DINV[:])
            w1_t = rpool.tile([F_IN, HID], f32)
            nc.sync.dma_start(w1_t[:], W1T[:])
            b1_t = rpool.tile([128, HID], f32)
            nc.sync.dma_start(b1_t[:], B1T[:])
            w2_t = rpool.tile([HID, OUT_D], f32)
            nc.sync.dma_start(w2_t[:], W2T[:])
            b2_t = rpool.tile([128, OUT_D], f32)
            nc.sync.dma_start(b2_t[:], B2T[:])
            ident = rpool.tile([128, 128], bf16)
            make_identity(nc, ident[:])

            ht_t = rpool.tile([HID, NG * 128], bf16)
            oute_t = rpool.tile([128, NG * OUT_D], f32)

            dma_sems = [nc.alloc_semaphore(f"gdma{q}") for q in range(4)]

            def issue_gathers(table, prep_n=0, gate_name=None, snif=None):
                msg_tiles = {}
                entries = []
                kmax = max(len(cs) for cs in chunks)
                for k in range(kmax):
                    for s in range(NSEC):
                        if k < len(chunks[s]):
                            entries.append((s, k, chunks[s][k]))
                for i, (s, k, csz) in enumerate(entries):
                    qn = i % 4
                    ng = csz // 128
                    msg = mpool.tile([128, CHUNK // 128, F_IN], bf16, tag="msg")
                    kw = {}
                    if i < prep_n:
                        kw = dict(prepare_only=True, sem=dma_sems[qn])
                    h = nc.gpsimd.dma_gather(
                        msg[:, :ng, :],
                        table[s * SEC:(s + 1) * SEC, :],
                        idx_t[:, sec_coff[s] + k * (CHUNK // 16):
                              sec_coff[s] + k * (CHUNK // 16) + csz // 16],
                        csz,
                        csz,
                        F_IN,
                        single_packet=True,
                        queue_num=qn,
                        **kw,
                    )
                    if i < prep_n and gate_name is not None:
                        # Desc-gen only reads idx; the table read happens at
                        # trigger time (gated via the sniffer WAW below), so
                        # demote the collective -> prep RAW edge to no-sync.
                        if h.ins.try_remove_dependency(gate_name):
                            ns = InstructionNameOrderedSet()
                            ns.add(gate_name)
                            h.ins.add_nosync_dependencies_from(ns)
                    if i == prep_n - 1:
                        for q in range(4):
                            nc.gpsimd.trigger_dma(
                                count=None, queue_num=q,
                                signals_writable=(
                                    [snif[:]] if snif is not None else ()),
                            )
                    msg_tiles[(s, k)] = msg
                return msg_tiles

            def scatter_block(b, msg_tiles, macc, mwid):
                gblk = sum(len(groups[b][s]) for s in range(NSEC))
                g0 = sum(sum(len(groups[bb][s]) for s in range(NSEC))
                         for bb in range(b))
                s_t = spool.tile([128, GBLK_MAX, 128], fp8, tag="sval")
                nc.sync.dma_start(s_t[:, :gblk, :], SVAL[:, g0:g0 + gblk, :])
                acc = apool.tile([128, BCOLS], f32, tag="acc")
                mm = 0
                for s in range(NSEC):
                    for (tk, jj, base) in groups[b][s]:
                        nc.tensor.matmul(
                            acc[:macc, base:base + 128],
                            msg_tiles[(s, tk)][:, jj, :mwid],
                            s_t[:, mm, :],
                            start=(mm == 0),
                            stop=(mm == gblk - 1),
                        )
                        mm += 1
                return acc

            # ---------------- layer 1 ----------------
            msg_tiles = issue_gathers(T1)
            for b in range(NBLK):
                blo = b * BCOLS
                blen = min(BCOLS, NPC - blo)
                acc = scatter_block(b, msg_tiles, 128, 128)
                gcols = _round128(blen)
                t1d_b = ypool.tile([128, BCOLS], bf16, tag="t1d")
                nc.sync.dma_start(t1d_b[:, :gcols], T1D[:, blo:blo + gcols])
                hb = ppool.tile([128, BCOLS], f32, tag="hb1")
                nc.vector.tensor_tensor(
                    out=hb[:, :blen], in0=acc[:, :blen],
                    in1=t1d_b[:, :blen], op=mybir.AluOpType.add)
                if blen < BCOLS:
                    nc.vector.memset(hb[:, blen:_round128(blen)], 0.0)
                tloc_b = ppool.tile([128, BCOLS], bf16, tag="tloc")
                nc.vector.memset(tloc_b[:], 0.0)
                for g in range(blo // 128, (blo + blen + 127) // 128):
                    go = g * 128 - blo
                    pt = tpool.tile([128, 128], f32, tag="t")
                    nc.tensor.matmul(
                        pt[:, :HID], hb[:, go:go + 128], w1_t[:],
                        start=True, stop=True)
                    tmp = ypool.tile([128, HID], f32, tag="tmp")
                    nc.vector.tensor_scalar(
                        out=tmp[:], in0=pt[:, :HID], scalar1=dinv_t[:, g:g + 1],
                        scalar2=None, op0=mybir.AluOpType.mult)
                    nc.vector.tensor_tensor(
                        out=tmp[:], in0=tmp[:], in1=b1_t[:],
                        op=mybir.AluOpType.add)
                    # relu + src-side dinv pre-scale, written into the table
                    plim = 84 if g == NG - 1 else 128
                    nc.vector.tensor_scalar(
                        out=tloc_b[:plim, go:go + HID],
                        in0=tmp[:plim, :], scalar1=0.0,
                        scalar2=dinv_t[:plim, g:g + 1],
                        op0=mybir.AluOpType.max, op1=mybir.AluOpType.mult)
                nc.sync.dma_start(
                    t_local[:, blo:blo + gcols], tloc_b[:, :gcols])

            cc = nc.gpsimd.collective_compute(
                "AllGather",
                mybir.AluOpType.bypass,
                replica_groups=[list(range(NCORES))],
                ins=[t_local[:]],
                outs=[t_full[:]],
            )
            # Build the transposed hidden table (layer-2 diagonal term) from
            # t_local during the collective -- PE/DVE are idle in that window.
            tl_t = rpool.tile([128, NG * F_IN], bf16)
            nc.sync.dma_start(tl_t[:], t_local[:])
            for g in range(NG):
                pt2 = tpool.tile([128, 128], bf16, tag="tb")
                nc.tensor.transpose(
                    pt2[:HID, :], tl_t[:, g * F_IN:g * F_IN + HID], ident[:])
                nc.vector.tensor_copy(
                    ht_t[:, g * 128:(g + 1) * 128], pt2[:HID, :])
            snif = None

            # ---------------- layer 2 ----------------
            msg_tiles = issue_gathers(t_full, prep_n=PREPN,
                                      gate_name=cc.ins.name, snif=snif)
            for b in range(NBLK):
                blo = b * BCOLS
                blen = min(BCOLS, NPC - blo)
                acc = scatter_block(b, msg_tiles, HID, HID)
                hb = ppool.tile([HID, BCOLS], f32, tag="hb2")
                nc.vector.tensor_tensor(
                    out=hb[:, :blen], in0=acc[:HID, :blen],
                    in1=ht_t[:, blo:blo + blen], op=mybir.AluOpType.add)
                if blen < BCOLS:
                    nc.vector.memset(hb[:, blen:_round128(blen)], 0.0)
                for g in range(blo // 128, (blo + blen + 127) // 128):
                    go = g * 128 - blo
                    pt = tpool.tile([128, 128], f32, tag="t")
                    nc.tensor.matmul(
                        pt[:, :OUT_D], hb[:, go:go + 128], w2_t[:],
                        start=True, stop=True)
                    ot = ypool.tile([128, OUT_D], f32, tag="ot")
                    nc.vector.tensor_scalar(
                        out=ot[:], in0=pt[:, :OUT_D], scalar1=dinv_t[:, g:g + 1],
                        scalar2=None, op0=mybir.AluOpType.mult)
                    nc.vector.tensor_tensor(
                        out=oute_t[:, g * OUT_D:(g + 1) * OUT_D],
                        in0=ot[:], in1=b2_t[:], op=mybir.AluOpType.add)

            nc.sync.dma_start(OUTE[:], oute_t[:])

    nc.compile()
    return nc


# ============================ entry point ============================

def prepare(x, edge_index, W1, b1, W2, b2):
    x = np.asarray(x, dtype=np.float32)
    W1 = np.asarray(W1, dtype=np.float32)
    b1 = np.asarray(b1, dtype=np.float32)
    W2 = np.asarray(W2, dtype=np.float32)
    b2 = np.asarray(b2, dtype=np.float32)

    sched, idx_all, sval_all, dinv_gt, dinv = _host_prep(edge_index)

    key = ("v2", CHUNK, sched["CIDX"], sched["TG"])
    if key in _CACHE:
        nc = _CACHE[key]
    else:
        nc = build_program(sched)
        _CACHE[key] = nc

    xs = x * dinv[:, None]
    T1 = np.zeros((NTOT, F_IN), dtype=ml_dtypes.bfloat16)
    n = np.arange(N)
    sl = n % NPC
    srow = (n // NPC) * SH + (sl % 128) * NG + (sl // 128)
    T1[srow] = xs.astype(ml_dtypes.bfloat16)

    b1_tile = np.tile(b1[None, :], (128, 1)).astype(np.float32)
    b2_tile = np.tile(b2[None, :], (128, 1)).astype(np.float32)

    in_maps = []
    for c in range(NCORES):
        t1d = np.zeros((128, NG * 128), dtype=ml_dtypes.bfloat16)
        xsl = np.zeros((SH, F_IN), dtype=np.float32)
        xsl[:NPC] = xs[c * NPC:(c + 1) * NPC]
        # t1d[f, l] = xs[c*NPC + l][f]
        t1d[:, :] = xsl.T.astype(ml_dtypes.bfloat16)
        in_maps.append({
            "T1": T1,
            "IDX": np.ascontiguousarray(idx_all[c]),
            "SVAL": np.ascontiguousarray(sval_all[c]),
            "DINV": np.ascontiguousarray(dinv_gt[c]),
            "T1D": t1d,
            "W1T": W1,
            "B1T": b1_tile,
            "W2T": W2,
            "B2T": b2_tile,
        })
    return nc, in_maps


def assemble_output(results):
    out = np.empty((N, OUT_D), dtype=np.float32)
    for c in range(NCORES):
        oc = np.asarray(results[c]["OUTE"]).reshape(128, NG, OUT_D)
        out[c * NPC:(c + 1) * NPC] = (
            oc.transpose(1, 0, 2).reshape(SH, OUT_D)[:NPC])
    return out


def kernel(x, edge_index, W1, b1, W2, b2):
    from concourse.bass_utils import run_bass_kernel_spmd

    nc, in_maps = prepare(x, edge_index, W1, b1, W2, b2)
    r = run_bass_kernel_spmd(nc, in_maps, core_ids=list(range(NCORES)))
    return assemble_output(r.results)
